# revision 10
# baseline (speedup 1.0000x reference)
"""Trainium2 Bass kernel for nn_DecoderLayer (dense transformer decoder layer).

Sharding: 8 cores = 4 batches x 2 query-halves (512 query rows each).
Uniform SPMD program across cores; causality enters via per-core 0/1 mask
*data* (multiplied after exp), not program structure.

All matmuls run as fp32r (fp32 bits on the full-rate PE path) with N=512.
Attention uses transposed scores S^T[k,q] so no P-transpose is ever needed:
  y^T = LN(x)^T (PE transpose),  Q^T/K^T = W^T-stationary projections,
  S^T[k,q] = K^T-chunk-stationary @ Q^T,  P^T = exp(S^T*scale)*mask,
  sums = ones-column matmul,  O^T[e,q] = V-stationary @ P^T (accumulated raw),
  normalize O^T by PE-outer-product broadcast of 1/sums,
  C^T[d,q] = Wc^T-stationary @ O^T accumulated over heads.
Residual stream kept in N layout [tokens, d]; PE transposes convert.

Self-contained: imports only /opt/trn_rl_repo + numpy/jax.
"""
import sys
sys.path.insert(0, '/opt/trn_rl_repo')

import math
import numpy as np

B, L, D, H, E, F = 4, 1024, 512, 8, 512, 2048
EPS = 1e-5
SCALE = 1.0 / math.sqrt(D)
LQ = 512
NCORES = 8
DC = D // 128
ECH = E // 128
LT = L // 128
QT = LQ // 128
FC = F // 128
WCC = (H * E) // 128

PER_CORE_INPUTS = ("dec_full", "dec_own", "encT", "mask")

_CACHE = {}


# --------------------------------------------------------------------------
# device program
# --------------------------------------------------------------------------
def build_nc(debug=False, n_iter=0):
    from contextlib import ExitStack
    from concourse import bacc, mybir, tile
    from concourse.masks import make_identity

    F32 = mybir.dt.float32
    F32R = mybir.dt.float32r
    AF = mybir.ActivationFunctionType
    OP = mybir.AluOpType

    nc = bacc.Bacc("TRN2", target_bir_lowering=False, debug=False)

    def din(name, shape, dt=F32R):
        return nc.dram_tensor(name, shape, dt, kind="ExternalInput").ap()

    dec_full = din("dec_full", [LT, 128, D])
    dec_own = din("dec_own", [QT, 128, D])
    encT_d = din("encT", [DC, 128, L])
    mask_d = din("mask", [128, LT, LQ])
    w_in = {}
    for p in ("sa", "ca"):
        w_in[p + "_wqT"] = din(p + "_wqT", [H, DC, 128, E])
        w_in[p + "_wkT"] = din(p + "_wkT", [H, DC, 128, E])
        w_in[p + "_wvT"] = din(p + "_wvT", [H, DC, 128, E])
        w_in[p + "_wcT"] = din(p + "_wcT", [WCC, 128, D])
        w_in[p + "_bqc"] = din(p + "_bqc", [128, H, ECH], F32)
        w_in[p + "_bkc"] = din(p + "_bkc", [128, H, ECH], F32)
        w_in[p + "_bv"] = din(p + "_bv", [H, E])
        w_in[p + "_bcc"] = din(p + "_bcc", [128, DC], F32)
    ff_w1T = din("ff_w1T", [DC, 128, F])
    ff_w2T = din("ff_w2T", [FC, 128, D])
    ff_b1c = din("ff_b1c", [128, FC], F32)
    ff_b2c = din("ff_b2c", [128, DC], F32)
    lnc = {k: din(k, [128, DC], F32) for k in
           ("ln1_gc", "ln1_bc", "ln2_gc", "ln2_bc", "ln3_gc", "ln3_bc")}

    out = nc.dram_tensor("out", [QT, 128, D], F32, kind="ExternalOutput").ap()
    dbg = {}
    if debug:
        for nm, shp in (("dbg_y1T", [128, DC, L]), ("dbg_kT", [128, ECH, L]),
                        ("dbg_pT", [128, LT, LQ]), ("dbg_oT", [128, ECH, LQ]),
                        ("dbg_x1", [QT, 128, D]), ("dbg_x2", [QT, 128, D])):
            dbg[nm] = nc.dram_tensor(nm, shp, F32, kind="ExternalOutput").ap()

    with ExitStack() as ctx:
        ctx.enter_context(nc.allow_low_precision(reason="float32r has float32 bits"))
        tc = ctx.enter_context(tile.TileContext(nc))
        per = ctx.enter_context(tc.tile_pool(name="per", bufs=1))
        ps = ctx.enter_context(tc.tile_pool(name="ps", bufs=1, space="PSUM"))

        ident = per.tile([128, 128], F32)
        make_identity(nc, ident)
        ones_f32 = per.tile([128, 1], F32)
        nc.vector.memset(ones_f32, 1.0)
        ones_k = per.tile([128, 1], F32R)
        nc.vector.tensor_copy(out=ones_k, in_=ones_f32)
        ones_r = per.tile([1, 128], F32R)
        nc.vector.tensor_copy(out=ones_r, in_=ones_f32[0:1, 0:1].to_broadcast((1, 128)))
        eps_t = per.tile([128, 1], F32)
        nc.vector.memset(eps_t, EPS)
        zero_t = per.tile([128, 1], F32)
        nc.vector.memset(zero_t, 0.0)

        def load_small(ap_in, shape, nm):
            t = per.tile(shape, F32, name=nm)
            nc.sync.dma_start(out=t, in_=ap_in)
            return t

        biasq = {p: load_small(w_in[p + "_bqc"], [128, H, ECH], p + "bq") for p in ("sa", "ca")}
        biask = {p: load_small(w_in[p + "_bkc"], [128, H, ECH], p + "bk") for p in ("sa", "ca")}
        biasc = {p: load_small(w_in[p + "_bcc"], [128, DC], p + "bc") for p in ("sa", "ca")}
        lng = {k: load_small(v, [128, DC], k) for k, v in lnc.items()}
        b1t = load_small(ff_b1c, [128, FC], "b1t")
        b2t = load_small(ff_b2c, [128, DC], "b2t")

        enc_t = per.tile([128, DC, L], F32R)
        for dc in range(DC):
            nc.sync.dma_start(out=enc_t[:, dc, :], in_=encT_d[dc])

        def body():
            pool = tc.alloc_tile_pool(name="work", bufs=1)
            sa_pool = tc.alloc_tile_pool(name="sa_only", bufs=1)

            mask_t = sa_pool.tile([128, LT, LQ], F32R)
            for kt in range(LT):
                nc.sync.dma_start(out=mask_t[:, kt, :], in_=mask_d[:, kt, :])
            y1T = sa_pool.tile([128, DC, L], F32R)

            y1To = pool.tile([128, DC, LQ], F32R, tag="yTo")
            x1 = pool.tile([128, QT, D], F32, tag="x1")
            x2 = pool.tile([128, QT, D], F32, tag="x2")

            # ---------- LayerNorm -> transposed output ----------
            def ln_to_T(src_dram, n_tiles, yT_dst, gk, bk, x_sbuf=None):
                for it in range(n_tiles):
                    if x_sbuf is None:
                        xt = pool.tile([128, D], F32, name="ln_x", tag="ln_x", bufs=2)
                        nc.sync.dma_start(out=xt, in_=src_dram[it].bitcast(F32))
                    else:
                        xt = x_sbuf[:, it, :]
                    stats = pool.tile([128, 6], F32, name="ln_s", tag="ln_s", bufs=2)
                    nc.vector.bn_stats(out=stats, in_=xt)
                    mv = pool.tile([128, 2], F32, name="ln_mv", tag="ln_mv", bufs=2)
                    nc.vector.bn_aggr(out=mv, in_=stats)
                    std = pool.tile([128, 1], F32, name="ln_d", tag="ln_d", bufs=2)
                    nc.scalar.activation(out=std, in_=mv[:, 1:2], func=AF.Sqrt,
                                         bias=eps_t, scale=1.0)
                    rstd = pool.tile([128, 1], F32, name="ln_r", tag="ln_r", bufs=2)
                    nc.vector.reciprocal(out=rstd, in_=std)
                    yn = pool.tile([128, D], F32, name="ln_y", tag="ln_y", bufs=2)
                    nc.vector.tensor_scalar(out=yn, in0=xt, scalar1=mv[:, 0:1],
                                            scalar2=rstd, op0=OP.subtract, op1=OP.mult)
                    tp = ps.tile([128, DC, 128], F32, name="ps_tp", tag="s", bufs=3)
                    for dc in range(DC):
                        nc.tensor.transpose(tp[:, dc, :],
                                            yn[:, dc * 128:(dc + 1) * 128], ident)
                    for dc in range(DC):
                        nc.vector.tensor_scalar(
                            out=yT_dst[:, dc, it * 128:(it + 1) * 128],
                            in0=tp[:, dc, :], scalar1=lng[gk][:, dc:dc + 1],
                            scalar2=lng[bk][:, dc:dc + 1], op0=OP.mult, op1=OP.add)

            # ---------- attention ----------
            def attention(prefix, pool_a, qT_own, kvT, kv_len, use_mask, cacc):
                nkt = kv_len // 128
                for h in range(H):
                    wq_t = pool_a.tile([128, DC, E], F32R, name="wq", tag="wq", bufs=2)
                    wk_t = pool_a.tile([128, DC, E], F32R, name="wk", tag="wk", bufs=2)
                    wv_t = pool_a.tile([128, DC, E], F32R, name="wv", tag="wv", bufs=2)
                    for dc in range(DC):
                        nc.sync.dma_start(out=wq_t[:, dc, :], in_=w_in[prefix + "_wqT"][h, dc])
                        nc.sync.dma_start(out=wk_t[:, dc, :], in_=w_in[prefix + "_wkT"][h, dc])
                        nc.sync.dma_start(out=wv_t[:, dc, :], in_=w_in[prefix + "_wvT"][h, dc])
                    wc_t = pool_a.tile([128, ECH, D], F32R, name="wc", tag="wc", bufs=1)
                    for ec in range(ECH):
                        nc.sync.dma_start(out=wc_t[:, ec, :],
                                          in_=w_in[prefix + "_wcT"][h * ECH + ec])
                    bv_t = pool_a.tile([128, E], F32R, name="bv", tag="bv", bufs=2)
                    nc.sync.dma_start(
                        out=bv_t,
                        in_=w_in[prefix + "_bv"][h:h + 1, :].to_broadcast((128, E)))

                    qT = pool_a.tile([128, ECH, LQ], F32R, name="qT", tag="qT", bufs=1)
                    kT = pool_a.tile([128, ECH, kv_len], F32R, name="kT", tag="kT", bufs=1)
                    for ec in range(ECH):
                        accq = ps.tile([128, LQ], F32, name="ps_q", tag="s", bufs=3)
                        for dc in range(DC):
                            nc.tensor.matmul(accq,
                                             lhsT=wq_t[:, dc, ec * 128:(ec + 1) * 128],
                                             rhs=qT_own[:, dc, :],
                                             start=dc == 0, stop=dc == DC - 1)
                        nc.vector.tensor_scalar_add(
                            out=qT[:, ec, :], in0=accq,
                            scalar1=biasq[prefix][:, h, ec:ec + 1])
                        for lt in range(kv_len // 512):
                            acck = ps.tile([128, 512], F32, name="ps_k", tag="s", bufs=3)
                            for dc in range(DC):
                                nc.tensor.matmul(
                                    acck, lhsT=wk_t[:, dc, ec * 128:(ec + 1) * 128],
                                    rhs=kvT[:, dc, lt * 512:(lt + 1) * 512],
                                    start=dc == 0, stop=dc == DC - 1)
                            nc.vector.tensor_scalar_add(
                                out=kT[:, ec, lt * 512:(lt + 1) * 512], in0=acck,
                                scalar1=biask[prefix][:, h, ec:ec + 1])

                    # streaming over key tiles: V jit, scores, exp, mask, sums, AV
                    sums = ps.tile([1, LQ], F32, name="ps_sum", tag="sum", bufs=1)
                    po = [ps.tile([128, LQ], F32, name=f"ps_o{ec}", tag="big", bufs=4)
                          for ec in range(ECH)]
                    for kt in range(nkt):
                        accv = ps.tile([128, E], F32, name="ps_v", tag="s", bufs=3)
                        for dc in range(DC):
                            nc.tensor.matmul(accv,
                                             lhsT=kvT[:, dc, kt * 128:(kt + 1) * 128],
                                             rhs=wv_t[:, dc, :],
                                             start=dc == 0, stop=dc == DC - 1)
                        vt = pool_a.tile([128, E], F32R, name="vt", tag="vt", bufs=3)
                        nc.vector.tensor_add(out=vt, in0=accv, in1=bv_t)

                        sc = ps.tile([128, LQ], F32, name="ps_s", tag="s", bufs=3)
                        for ec in range(ECH):
                            nc.tensor.matmul(sc,
                                             lhsT=kT[:, ec, kt * 128:(kt + 1) * 128],
                                             rhs=qT[:, ec, :],
                                             start=ec == 0, stop=ec == ECH - 1)
                        pt = pool_a.tile([128, LQ], F32R, name="pt", tag="pt", bufs=3)
                        nc.scalar.activation(out=pt, in_=sc, func=AF.Exp, scale=SCALE,
                                             bias=zero_t)
                        if use_mask:
                            nc.vector.tensor_mul(out=pt, in0=pt, in1=mask_t[:, kt, :])
                        nc.tensor.matmul(sums, lhsT=ones_k, rhs=pt,
                                         start=kt == 0, stop=kt == nkt - 1)
                        for ec in range(ECH):
                            nc.tensor.matmul(po[ec],
                                             lhsT=vt[:, ec * 128:(ec + 1) * 128],
                                             rhs=pt, start=kt == 0, stop=kt == nkt - 1)
                        if debug and prefix == "sa" and h == 0:
                            dt1 = pool.tile([128, LQ], F32, name="dbg1", tag="dbg", bufs=2)
                            nc.vector.tensor_copy(out=dt1, in_=pt.bitcast(F32))
                            nc.sync.dma_start(out=dbg["dbg_pT"][:, kt, :], in_=dt1)

                    recip = pool_a.tile([1, LQ], F32R, name="recip", tag="recip", bufs=1)
                    nc.vector.reciprocal(out=recip, in_=sums)
                    rb_ps = ps.tile([128, LQ], F32, name="ps_rb", tag="s", bufs=3)
                    nc.tensor.matmul(rb_ps, lhsT=ones_r, rhs=recip, start=True, stop=True)
                    rb = pool_a.tile([128, LQ], F32, name="rb", tag="rb", bufs=1)
                    nc.vector.tensor_copy(out=rb, in_=rb_ps)

                    oT = pool_a.tile([128, ECH, LQ], F32R, name="oT", tag="oT", bufs=1)
                    for ec in range(ECH):
                        nc.vector.tensor_mul(out=oT[:, ec, :], in0=po[ec], in1=rb)
                    for dc in range(DC):
                        pc = ps.tile([128, LQ], F32, name="ps_c", tag="big", bufs=4)
                        for ec in range(ECH):
                            nc.tensor.matmul(pc,
                                             lhsT=wc_t[:, ec, dc * 128:(dc + 1) * 128],
                                             rhs=oT[:, ec, :],
                                             start=ec == 0, stop=ec == ECH - 1)
                        if h == 0:
                            nc.vector.tensor_copy(out=cacc[:, dc, :], in_=pc)
                        else:
                            nc.vector.tensor_add(out=cacc[:, dc, :],
                                                 in0=cacc[:, dc, :], in1=pc)
                    if debug and prefix == "sa" and h == 0:
                        for ec in range(ECH):
                            dt2 = pool.tile([128, LQ], F32, name="dbg2", tag="dbg", bufs=2)
                            nc.vector.tensor_copy(out=dt2, in_=oT[:, ec, :].bitcast(F32))
                            nc.sync.dma_start(out=dbg["dbg_oT"][:, ec, :], in_=dt2)
                        for ec in range(ECH):
                            for lt in range(kv_len // 512):
                                dt3 = pool.tile([128, 512], F32, name="dbg3", tag="dbg", bufs=2)
                                nc.vector.tensor_copy(
                                    out=dt3,
                                    in_=kT[:, ec, lt * 512:(lt + 1) * 512].bitcast(F32))
                                nc.sync.dma_start(
                                    out=dbg["dbg_kT"][:, ec, lt * 512:(lt + 1) * 512],
                                    in_=dt3)

            # ---------- epilogue: xout(N) = base(N) + transpose(cacc + bc) ----------
            def epilogue(cacc, bcc_t, base_dram, base_sbuf, xout):
                for dc in range(DC):
                    nc.vector.tensor_scalar_add(out=cacc[:, dc, :], in0=cacc[:, dc, :],
                                                scalar1=bcc_t[:, dc:dc + 1])
                for qt in range(QT):
                    tp = ps.tile([128, DC, 128], F32, name="ps_ep", tag="s", bufs=3)
                    for dc in range(DC):
                        nc.tensor.transpose(tp[:, dc, :],
                                            cacc[:, dc, qt * 128:(qt + 1) * 128], ident)
                    tpf = tp.rearrange("p a b -> p (a b)")
                    if base_sbuf is None:
                        bt = pool.tile([128, D], F32, name="ep_b", tag="ln_x", bufs=2)
                        nc.sync.dma_start(out=bt, in_=base_dram[qt].bitcast(F32))
                        nc.vector.tensor_add(out=xout[:, qt, :], in0=tpf, in1=bt)
                    else:
                        nc.vector.tensor_add(out=xout[:, qt, :], in0=tpf,
                                             in1=base_sbuf[:, qt, :])

            # ================= pass =================
            ln_to_T(dec_full, LT, y1T, "ln1_gc", "ln1_bc")
            ln_to_T(dec_own, QT, y1To, "ln1_gc", "ln1_bc")
            if debug:
                for dc in range(DC):
                    for lt in range(2):
                        dt0 = pool.tile([128, 512], F32, name="dbg0", tag="dbg", bufs=2)
                        nc.vector.tensor_copy(
                            out=dt0, in_=y1T[:, dc, lt * 512:(lt + 1) * 512].bitcast(F32))
                        nc.sync.dma_start(
                            out=dbg["dbg_y1T"][:, dc, lt * 512:(lt + 1) * 512], in_=dt0)

            attn1 = tc.alloc_tile_pool(name="attn1", bufs=1)
            cacc1 = pool.tile([128, DC, LQ], F32, name="cacc1", tag="cacc")
            attention("sa", attn1, y1To, y1T, L, True, cacc1)
            epilogue(cacc1, biasc["sa"], dec_own, None, x1)
            attn1.release()
            sa_pool.release()
            if debug:
                for qt in range(QT):
                    nc.sync.dma_start(out=dbg["dbg_x1"][qt], in_=x1[:, qt, :])

            y2To = pool.tile([128, DC, LQ], F32R, name="y2To", tag="yTo")
            ln_to_T(None, QT, y2To, "ln2_gc", "ln2_bc", x_sbuf=x1)
            attn2 = tc.alloc_tile_pool(name="attn2", bufs=1)
            cacc2 = pool.tile([128, DC, LQ], F32, name="cacc2", tag="cacc")
            attention("ca", attn2, y2To, enc_t, L, False, cacc2)
            epilogue(cacc2, biasc["ca"], None, x1, x2)
            attn2.release()
            if debug:
                for qt in range(QT):
                    nc.sync.dma_start(out=dbg["dbg_x2"][qt], in_=x2[:, qt, :])

            y3To = pool.tile([128, DC, LQ], F32R, name="y3To", tag="yTo")
            ln_to_T(None, QT, y3To, "ln3_gc", "ln3_bc", x_sbuf=x2)

            # ---------- FFN ----------
            ffn = tc.alloc_tile_pool(name="ffn", bufs=1)
            w1_t = ffn.tile([128, DC, F], F32R)
            for dc in range(DC):
                nc.sync.dma_start(out=w1_t[:, dc, :], in_=ff_w1T[dc])
            w2_t = ffn.tile([128, FC, D], F32R)
            for fc in range(FC):
                nc.sync.dma_start(out=w2_t[:, fc, :], in_=ff_w2T[fc])
            uT = ffn.tile([128, FC, LQ], F32R)
            for fc in range(FC):
                pu = ps.tile([128, LQ], F32, name="ps_u", tag="s", bufs=3)
                for dc in range(DC):
                    nc.tensor.matmul(pu, lhsT=w1_t[:, dc, fc * 128:(fc + 1) * 128],
                                     rhs=y3To[:, dc, :], start=dc == 0, stop=dc == DC - 1)
                a = ffn.tile([128, LQ], F32, name="ff_a", tag="ff_a", bufs=2)
                nc.vector.tensor_scalar_add(out=a, in0=pu, scalar1=b1t[:, fc:fc + 1])
                mn = ffn.tile([128, LQ], F32, name="ff_m", tag="ff_m", bufs=2)
                nc.vector.tensor_scalar_min(out=mn, in0=a, scalar1=0.0)
                ex = ffn.tile([128, LQ], F32, name="ff_e", tag="ff_e", bufs=2)
                nc.scalar.activation(out=ex, in_=mn, func=AF.Exp, bias=zero_t)
                p1 = ffn.tile([128, LQ], F32, name="ff_p", tag="ff_p", bufs=2)
                nc.vector.tensor_scalar(out=p1, in0=a, scalar1=0.0, scalar2=-1.0,
                                        op0=OP.max, op1=OP.add)
                nc.vector.tensor_add(out=uT[:, fc, :], in0=ex, in1=p1)
            fT = ffn.tile([128, DC, LQ], F32)
            for dc in range(DC):
                pf = ps.tile([128, LQ], F32, name="ps_f", tag="big", bufs=4)
                for fc in range(FC):
                    nc.tensor.matmul(pf, lhsT=w2_t[:, fc, dc * 128:(dc + 1) * 128],
                                     rhs=uT[:, fc, :], start=fc == 0, stop=fc == FC - 1)
                nc.vector.tensor_scalar_add(out=fT[:, dc, :], in0=pf,
                                            scalar1=b2t[:, dc:dc + 1])
            for qt in range(QT):
                tp = ps.tile([128, DC, 128], F32, name="ps_ft", tag="s", bufs=3)
                for dc in range(DC):
                    nc.tensor.transpose(tp[:, dc, :],
                                        fT[:, dc, qt * 128:(qt + 1) * 128], ident)
                ot = ffn.tile([128, D], F32, name="out_t", tag="out_t", bufs=2)
                nc.vector.tensor_add(out=ot, in0=tp.rearrange("p a b -> p (a b)"),
                                     in1=x2[:, qt, :])
                nc.sync.dma_start(out=out[qt], in_=ot)
            ffn.release()
            pool.release()

        if n_iter > 0:
            with tc.For_i(0, n_iter, 1):
                body()
        else:
            body()

    nc.compile()
    return nc


# --------------------------------------------------------------------------
# host side
# --------------------------------------------------------------------------
def _prep_shared(params):
    P = {k: np.ascontiguousarray(np.asarray(v, dtype=np.float32))
         for k, v in params.items()}
    s = {}
    for p in ("sa", "ca"):
        s[p + "_wqT"] = np.ascontiguousarray(
            P[p + "_Wq"].transpose(0, 2, 1)).reshape(H, DC, 128, E)
        s[p + "_wkT"] = np.ascontiguousarray(
            P[p + "_Wk"].transpose(0, 2, 1)).reshape(H, DC, 128, E)
        s[p + "_wvT"] = np.ascontiguousarray(
            P[p + "_Wv"].transpose(0, 2, 1)).reshape(H, DC, 128, E)
        s[p + "_wcT"] = np.ascontiguousarray(P[p + "_Wc"].T).reshape(WCC, 128, D)
        s[p + "_bqc"] = np.ascontiguousarray(
            P[p + "_bq"].reshape(H, ECH, 128).transpose(2, 0, 1))
        s[p + "_bkc"] = np.ascontiguousarray(
            P[p + "_bk"].reshape(H, ECH, 128).transpose(2, 0, 1))
        s[p + "_bv"] = P[p + "_bv"]
        s[p + "_bcc"] = np.ascontiguousarray(P[p + "_bc"].reshape(DC, 128).T)
    s["ff_w1T"] = np.ascontiguousarray(P["ff_W1"].T).reshape(DC, 128, F)
    s["ff_w2T"] = np.ascontiguousarray(P["ff_W2"].T).reshape(FC, 128, D)
    s["ff_b1c"] = np.ascontiguousarray(P["ff_b1"].reshape(FC, 128).T)
    s["ff_b2c"] = np.ascontiguousarray(P["ff_b2"].reshape(DC, 128).T)
    for i in (1, 2, 3):
        s[f"ln{i}_gc"] = np.ascontiguousarray(P[f"ln{i}_g"].reshape(DC, 128).T)
        s[f"ln{i}_bc"] = np.ascontiguousarray(P[f"ln{i}_b"].reshape(DC, 128).T)
    return s


def _prep_percore(decoder_embeddings, encoder_embeddings):
    dec = np.asarray(decoder_embeddings, dtype=np.float32)
    enc = np.asarray(encoder_embeddings, dtype=np.float32)
    per = []
    k_global = (np.arange(128)[:, None] + 128 * np.arange(LT)[None, :])  # [128, LT]
    for c in range(NCORES):
        b, half = c // 2, c % 2
        q0 = half * LQ
        m = (k_global[:, :, None] <= (q0 + np.arange(LQ))[None, None, :])
        per.append({
            "dec_full": np.ascontiguousarray(dec[b].reshape(LT, 128, D)),
            "dec_own": np.ascontiguousarray(dec[b, q0:q0 + LQ].reshape(QT, 128, D)),
            "encT": np.ascontiguousarray(enc[b].T).reshape(DC, 128, L),
            "mask": np.ascontiguousarray(m.astype(np.float32)),
        })
    return per


def make_runner(nc):
    """Persistent jitted SPMD runner. Weights are replicated (no 8x concat)."""
    import jax
    from jax.sharding import Mesh, PartitionSpec
    from jax.experimental.shard_map import shard_map
    from concourse import mybir
    from concourse.bass2jax import (_bass_exec_p, install_neuronx_cc_hook,
                                    partition_id_tensor)
    install_neuronx_cc_hook()

    in_names, out_names, out_avals = [], [], []
    for alloc in nc.m.functions[0].allocations:
        if not isinstance(alloc, mybir.MemoryLocationSet):
            continue
        name = alloc.memorylocations[0].name
        if alloc.kind == "ExternalInput":
            in_names.append(name)
        elif alloc.kind == "ExternalOutput":
            out_names.append(name)
            out_avals.append(jax.core.ShapedArray(tuple(alloc.tensor_shape),
                                                  mybir.dt.np(alloc.dtype)))
    pname = nc.partition_id_tensor.name if nc.partition_id_tensor else None
    if pname:
        in_names.remove(pname)
    all_in = in_names + out_names + ([pname] if pname else [])

    def _body(*args):
        ops = list(args)
        if pname:
            ops.append(partition_id_tensor())
        return tuple(_bass_exec_p.bind(
            *ops, out_avals=tuple(out_avals), in_names=tuple(all_in),
            out_names=tuple(out_names), lowering_input_output_aliases=(),
            sim_require_finite=True, sim_require_nnan=True, nc=nc))

    mesh = Mesh(np.asarray(jax.devices()[:NCORES]), ("core",))
    in_specs = tuple(PartitionSpec("core") if n in PER_CORE_INPUTS else PartitionSpec()
                     for n in in_names) + (PartitionSpec("core"),) * len(out_names)
    out_specs = (PartitionSpec("core"),) * len(out_names)
    f = jax.jit(shard_map(_body, mesh=mesh, in_specs=in_specs, out_specs=out_specs,
                          check_rep=False), keep_unused=True)

    def run(shared, per_core, n_rep=1):
        import time
        args = []
        for n in in_names:
            if n in PER_CORE_INPUTS:
                args.append(np.concatenate([per_core[c][n] for c in range(NCORES)],
                                           axis=0))
            else:
                args.append(shared[n])
        for a in out_avals:
            args.append(np.zeros((NCORES * a.shape[0], *a.shape[1:]), a.dtype))
        args = [jax.device_put(a) for a in args]
        outs = f(*args)
        jax.block_until_ready(outs)
        walls = []
        for _ in range(max(0, n_rep - 1)):
            t0 = time.time()
            outs = f(*args)
            jax.block_until_ready(outs)
            walls.append(time.time() - t0)
        res = {nm: np.asarray(outs[i]).reshape(NCORES, *out_avals[i].shape)
               for i, nm in enumerate(out_names)}
        return res, walls

    return run


def _get(key, builder):
    if key not in _CACHE:
        _CACHE[key] = builder()
    return _CACHE[key]


def kernel(decoder_embeddings, encoder_embeddings, params):
    runner = _get("runner0", lambda: make_runner(_get("nc0", lambda: build_nc())))
    shared = _prep_shared(params)
    per_core = _prep_percore(decoder_embeddings, encoder_embeddings)
    res, _ = runner(shared, per_core)
    outs = res["out"]  # [8, QT, 128, D]
    full = np.empty((B, L, D), dtype=np.float32)
    for c in range(NCORES):
        b, half = c // 2, c % 2
        full[b, half * LQ:(half + 1) * LQ] = outs[c].reshape(LQ, D)
    return full


# revision 11
# speedup vs baseline: 1.0006x; 1.0006x over previous
"""Trainium2 Bass kernel for nn_DecoderLayer (dense transformer decoder layer).

Sharding: 8 cores = 4 batches x 2 query-halves (512 query rows each).
Uniform SPMD program across cores; causality enters via per-core 0/1 mask
*data* (multiplied after exp), not program structure.

All matmuls run as fp32r (fp32 bits on the full-rate PE path) with N=512.
Attention uses transposed scores S^T[k,q] so no P-transpose is ever needed:
  y^T = LN(x)^T (PE transpose),  Q^T/K^T = W^T-stationary projections,
  S^T[k,q] = K^T-chunk-stationary @ Q^T,  P^T = exp(S^T*scale)*mask,
  sums = ones-column matmul,  O^T[e,q] = V-stationary @ P^T (accumulated raw),
  normalize O^T by PE-outer-product broadcast of 1/sums,
  C^T[d,q] = Wc^T-stationary @ O^T accumulated over heads.
Residual stream kept in N layout [tokens, d]; PE transposes convert.

Self-contained: imports only /opt/trn_rl_repo + numpy/jax.
"""
import sys
sys.path.insert(0, '/opt/trn_rl_repo')

import math
import numpy as np

B, L, D, H, E, F = 4, 1024, 512, 8, 512, 2048
EPS = 1e-5
SCALE = 1.0 / math.sqrt(D)
LQ = 512
NCORES = 8
DC = D // 128
ECH = E // 128
LT = L // 128
QT = LQ // 128
FC = F // 128
WCC = (H * E) // 128

PER_CORE_INPUTS = ("dec_full", "dec_own", "encT", "mask")

_CACHE = {}


# --------------------------------------------------------------------------
# device program
# --------------------------------------------------------------------------
def build_nc(debug=False, n_iter=0):
    from contextlib import ExitStack
    from concourse import bacc, mybir, tile
    from concourse.masks import make_identity

    F32 = mybir.dt.float32
    F32R = mybir.dt.float32r
    AF = mybir.ActivationFunctionType
    OP = mybir.AluOpType

    nc = bacc.Bacc("TRN2", target_bir_lowering=False, debug=False)

    def din(name, shape, dt=F32R):
        return nc.dram_tensor(name, shape, dt, kind="ExternalInput").ap()

    dec_full = din("dec_full", [LT, 128, D])
    dec_own = din("dec_own", [QT, 128, D])
    encT_d = din("encT", [DC, 128, L])
    mask_d = din("mask", [128, LT, LQ])
    w_in = {}
    for p in ("sa", "ca"):
        w_in[p + "_wqT"] = din(p + "_wqT", [H, DC, 128, E])
        w_in[p + "_wkT"] = din(p + "_wkT", [H, DC, 128, E])
        w_in[p + "_wvT"] = din(p + "_wvT", [H, DC, 128, E])
        w_in[p + "_wcT"] = din(p + "_wcT", [WCC, 128, D])
        w_in[p + "_bqc"] = din(p + "_bqc", [128, H, ECH], F32)
        w_in[p + "_bkc"] = din(p + "_bkc", [128, H, ECH], F32)
        w_in[p + "_bv"] = din(p + "_bv", [H, E])
        w_in[p + "_bcc"] = din(p + "_bcc", [128, DC], F32)
    ff_w1T = din("ff_w1T", [DC, 128, F])
    ff_w2T = din("ff_w2T", [FC, 128, D])
    ff_b1c = din("ff_b1c", [128, FC], F32)
    ff_b2c = din("ff_b2c", [128, DC], F32)
    lnc = {k: din(k, [128, DC], F32) for k in
           ("ln1_gc", "ln1_bc", "ln2_gc", "ln2_bc", "ln3_gc", "ln3_bc")}

    out = nc.dram_tensor("out", [QT, 128, D], F32, kind="ExternalOutput").ap()
    dbg = {}
    if debug:
        for nm, shp in (("dbg_y1T", [128, DC, L]), ("dbg_kT", [128, ECH, L]),
                        ("dbg_pT", [128, LT, LQ]), ("dbg_oT", [128, ECH, LQ]),
                        ("dbg_x1", [QT, 128, D]), ("dbg_x2", [QT, 128, D])):
            dbg[nm] = nc.dram_tensor(nm, shp, F32, kind="ExternalOutput").ap()

    with ExitStack() as ctx:
        ctx.enter_context(nc.allow_low_precision(reason="float32r has float32 bits"))
        tc = ctx.enter_context(tile.TileContext(nc))
        per = ctx.enter_context(tc.tile_pool(name="per", bufs=1))
        ps = ctx.enter_context(tc.tile_pool(name="ps", bufs=1, space="PSUM"))

        ident = per.tile([128, 128], F32)
        make_identity(nc, ident)
        ones_f32 = per.tile([128, 1], F32)
        nc.vector.memset(ones_f32, 1.0)
        ones_k = per.tile([128, 1], F32R)
        nc.vector.tensor_copy(out=ones_k, in_=ones_f32)
        ones_r = per.tile([1, 128], F32R)
        nc.vector.tensor_copy(out=ones_r, in_=ones_f32[0:1, 0:1].to_broadcast((1, 128)))
        eps_t = per.tile([128, 1], F32)
        nc.vector.memset(eps_t, EPS)
        zero_t = per.tile([128, 1], F32)
        nc.vector.memset(zero_t, 0.0)

        def load_small(ap_in, shape, nm):
            t = per.tile(shape, F32, name=nm)
            nc.sync.dma_start(out=t, in_=ap_in)
            return t

        biasq = {p: load_small(w_in[p + "_bqc"], [128, H, ECH], p + "bq") for p in ("sa", "ca")}
        biask = {p: load_small(w_in[p + "_bkc"], [128, H, ECH], p + "bk") for p in ("sa", "ca")}
        biasc = {p: load_small(w_in[p + "_bcc"], [128, DC], p + "bc") for p in ("sa", "ca")}
        lng = {k: load_small(v, [128, DC], k) for k, v in lnc.items()}
        b1t = load_small(ff_b1c, [128, FC], "b1t")
        b2t = load_small(ff_b2c, [128, DC], "b2t")

        enc_t = per.tile([128, DC, L], F32R)
        for dc in range(DC):
            nc.sync.dma_start(out=enc_t[:, dc, :], in_=encT_d[dc])

        def body():
            pool = tc.alloc_tile_pool(name="work", bufs=1)
            sa_pool = tc.alloc_tile_pool(name="sa_only", bufs=1)

            mask_t = sa_pool.tile([128, LT, LQ], F32R)
            for kt in range(LT):
                nc.sync.dma_start(out=mask_t[:, kt, :], in_=mask_d[:, kt, :])
            y1T = sa_pool.tile([128, DC, L], F32R)

            y1To = pool.tile([128, DC, LQ], F32R, tag="yTo")
            x1 = pool.tile([128, QT, D], F32, tag="x1")
            x2 = pool.tile([128, QT, D], F32, tag="x2")

            # ---------- LayerNorm -> transposed output ----------
            def ln_to_T(src_dram, n_tiles, yT_dst, gk, bk, x_sbuf=None):
                for it in range(n_tiles):
                    if x_sbuf is None:
                        xt = pool.tile([128, D], F32, name="ln_x", tag="ln_x", bufs=2)
                        nc.sync.dma_start(out=xt, in_=src_dram[it].bitcast(F32))
                    else:
                        xt = x_sbuf[:, it, :]
                    stats = pool.tile([128, 6], F32, name="ln_s", tag="ln_s", bufs=2)
                    nc.vector.bn_stats(out=stats, in_=xt)
                    mv = pool.tile([128, 2], F32, name="ln_mv", tag="ln_mv", bufs=2)
                    nc.vector.bn_aggr(out=mv, in_=stats)
                    std = pool.tile([128, 1], F32, name="ln_d", tag="ln_d", bufs=2)
                    nc.scalar.activation(out=std, in_=mv[:, 1:2], func=AF.Sqrt,
                                         bias=eps_t, scale=1.0)
                    rstd = pool.tile([128, 1], F32, name="ln_r", tag="ln_r", bufs=2)
                    nc.vector.reciprocal(out=rstd, in_=std)
                    yn = pool.tile([128, D], F32, name="ln_y", tag="ln_y", bufs=2)
                    nc.vector.tensor_scalar(out=yn, in0=xt, scalar1=mv[:, 0:1],
                                            scalar2=rstd, op0=OP.subtract, op1=OP.mult)
                    tp = ps.tile([128, DC, 128], F32, name="ps_tp", tag="s", bufs=3)
                    for dc in range(DC):
                        nc.tensor.transpose(tp[:, dc, :],
                                            yn[:, dc * 128:(dc + 1) * 128], ident)
                    for dc in range(DC):
                        nc.vector.tensor_scalar(
                            out=yT_dst[:, dc, it * 128:(it + 1) * 128],
                            in0=tp[:, dc, :], scalar1=lng[gk][:, dc:dc + 1],
                            scalar2=lng[bk][:, dc:dc + 1], op0=OP.mult, op1=OP.add)

            # ---------- attention ----------
            def attention(prefix, pool_a, qT_own, kvT, kv_len, use_mask, cacc):
                nkt = kv_len // 128
                for h in range(H):
                    wq_t = pool_a.tile([128, DC, E], F32R, name="wq", tag="wq", bufs=2)
                    wk_t = pool_a.tile([128, DC, E], F32R, name="wk", tag="wk", bufs=2)
                    wv_t = pool_a.tile([128, DC, E], F32R, name="wv", tag="wv", bufs=2)
                    for dc in range(DC):
                        nc.sync.dma_start(out=wq_t[:, dc, :], in_=w_in[prefix + "_wqT"][h, dc])
                        nc.sync.dma_start(out=wk_t[:, dc, :], in_=w_in[prefix + "_wkT"][h, dc])
                        nc.sync.dma_start(out=wv_t[:, dc, :], in_=w_in[prefix + "_wvT"][h, dc])
                    wc_t = pool_a.tile([128, ECH, D], F32R, name="wc", tag="wc", bufs=1)
                    for ec in range(ECH):
                        nc.sync.dma_start(out=wc_t[:, ec, :],
                                          in_=w_in[prefix + "_wcT"][h * ECH + ec])
                    bv_t = pool_a.tile([128, E], F32R, name="bv", tag="bv", bufs=2)
                    nc.sync.dma_start(
                        out=bv_t,
                        in_=w_in[prefix + "_bv"][h:h + 1, :].to_broadcast((128, E)))

                    qT = pool_a.tile([128, ECH, LQ], F32R, name="qT", tag="qT", bufs=1)
                    kT = pool_a.tile([128, ECH, kv_len], F32R, name="kT", tag="kT", bufs=1)
                    for ec in range(ECH):
                        accq = ps.tile([128, LQ], F32, name="ps_q", tag="s", bufs=3)
                        for dc in range(DC):
                            nc.tensor.matmul(accq,
                                             lhsT=wq_t[:, dc, ec * 128:(ec + 1) * 128],
                                             rhs=qT_own[:, dc, :],
                                             start=dc == 0, stop=dc == DC - 1)
                        nc.vector.tensor_scalar_add(
                            out=qT[:, ec, :], in0=accq,
                            scalar1=biasq[prefix][:, h, ec:ec + 1])
                        for lt in range(kv_len // 512):
                            acck = ps.tile([128, 512], F32, name="ps_k", tag="s", bufs=3)
                            for dc in range(DC):
                                nc.tensor.matmul(
                                    acck, lhsT=wk_t[:, dc, ec * 128:(ec + 1) * 128],
                                    rhs=kvT[:, dc, lt * 512:(lt + 1) * 512],
                                    start=dc == 0, stop=dc == DC - 1)
                            nc.vector.tensor_scalar_add(
                                out=kT[:, ec, lt * 512:(lt + 1) * 512], in0=acck,
                                scalar1=biask[prefix][:, h, ec:ec + 1])

                    # streaming over key tiles: V jit, scores, exp, mask, sums, AV
                    psacc = pool_a.tile([128, LQ], F32, name="psacc", tag="psacc", bufs=1)
                    po = [ps.tile([128, LQ], F32, name=f"ps_o{ec}", tag="big", bufs=4)
                          for ec in range(ECH)]
                    for kt in range(nkt):
                        accv = ps.tile([128, E], F32, name="ps_v", tag="s", bufs=3)
                        for dc in range(DC):
                            nc.tensor.matmul(accv,
                                             lhsT=kvT[:, dc, kt * 128:(kt + 1) * 128],
                                             rhs=wv_t[:, dc, :],
                                             start=dc == 0, stop=dc == DC - 1)
                        vt = pool_a.tile([128, E], F32R, name="vt", tag="vt", bufs=4)
                        nc.vector.tensor_add(out=vt, in0=accv, in1=bv_t)

                        sc = ps.tile([128, LQ], F32, name="ps_s", tag="s", bufs=3)
                        for ec in range(ECH):
                            nc.tensor.matmul(sc,
                                             lhsT=kT[:, ec, kt * 128:(kt + 1) * 128],
                                             rhs=qT[:, ec, :],
                                             start=ec == 0, stop=ec == ECH - 1)
                        pt = pool_a.tile([128, LQ], F32R, name="pt", tag="pt", bufs=4)
                        nc.scalar.activation(out=pt, in_=sc, func=AF.Exp, scale=SCALE,
                                             bias=zero_t)
                        if use_mask:
                            nc.vector.tensor_mul(out=pt, in0=pt, in1=mask_t[:, kt, :])
                        if kt == 0:
                            nc.vector.tensor_copy(out=psacc, in_=pt.bitcast(F32))
                        else:
                            nc.vector.tensor_add(out=psacc, in0=psacc, in1=pt.bitcast(F32))
                        for ec in range(ECH):
                            nc.tensor.matmul(po[ec],
                                             lhsT=vt[:, ec * 128:(ec + 1) * 128],
                                             rhs=pt, start=kt == 0, stop=kt == nkt - 1)
                        if debug and prefix == "sa" and h == 0:
                            dt1 = pool.tile([128, LQ], F32, name="dbg1", tag="dbg", bufs=2)
                            nc.vector.tensor_copy(out=dt1, in_=pt.bitcast(F32))
                            nc.sync.dma_start(out=dbg["dbg_pT"][:, kt, :], in_=dt1)

                    psacc_r = pool_a.tile([128, LQ], F32R, name="psacc_r", tag="psacc_r", bufs=1)
                    nc.vector.tensor_copy(out=psacc_r, in_=psacc)
                    sums = ps.tile([1, LQ], F32, name="ps_sum", tag="sum", bufs=1)
                    nc.tensor.matmul(sums, lhsT=ones_k, rhs=psacc_r, start=True, stop=True)
                    recip = pool_a.tile([1, LQ], F32R, name="recip", tag="recip", bufs=1)
                    nc.vector.reciprocal(out=recip, in_=sums)
                    rb_ps = ps.tile([128, LQ], F32, name="ps_rb", tag="s", bufs=3)
                    nc.tensor.matmul(rb_ps, lhsT=ones_r, rhs=recip, start=True, stop=True)
                    rb = pool_a.tile([128, LQ], F32, name="rb", tag="rb", bufs=1)
                    nc.vector.tensor_copy(out=rb, in_=rb_ps)

                    oT = pool_a.tile([128, ECH, LQ], F32R, name="oT", tag="oT", bufs=1)
                    for ec in range(ECH):
                        nc.vector.tensor_mul(out=oT[:, ec, :], in0=po[ec], in1=rb)
                    for dc in range(DC):
                        pc = ps.tile([128, LQ], F32, name="ps_c", tag="big", bufs=4)
                        for ec in range(ECH):
                            nc.tensor.matmul(pc,
                                             lhsT=wc_t[:, ec, dc * 128:(dc + 1) * 128],
                                             rhs=oT[:, ec, :],
                                             start=ec == 0, stop=ec == ECH - 1)
                        if h == 0:
                            nc.vector.tensor_copy(out=cacc[:, dc, :], in_=pc)
                        else:
                            nc.vector.tensor_add(out=cacc[:, dc, :],
                                                 in0=cacc[:, dc, :], in1=pc)
                    if debug and prefix == "sa" and h == 0:
                        for ec in range(ECH):
                            dt2 = pool.tile([128, LQ], F32, name="dbg2", tag="dbg", bufs=2)
                            nc.vector.tensor_copy(out=dt2, in_=oT[:, ec, :].bitcast(F32))
                            nc.sync.dma_start(out=dbg["dbg_oT"][:, ec, :], in_=dt2)
                        for ec in range(ECH):
                            for lt in range(kv_len // 512):
                                dt3 = pool.tile([128, 512], F32, name="dbg3", tag="dbg", bufs=2)
                                nc.vector.tensor_copy(
                                    out=dt3,
                                    in_=kT[:, ec, lt * 512:(lt + 1) * 512].bitcast(F32))
                                nc.sync.dma_start(
                                    out=dbg["dbg_kT"][:, ec, lt * 512:(lt + 1) * 512],
                                    in_=dt3)

            # ---------- epilogue: xout(N) = base(N) + transpose(cacc + bc) ----------
            def epilogue(cacc, bcc_t, base_dram, base_sbuf, xout):
                for dc in range(DC):
                    nc.vector.tensor_scalar_add(out=cacc[:, dc, :], in0=cacc[:, dc, :],
                                                scalar1=bcc_t[:, dc:dc + 1])
                for qt in range(QT):
                    tp = ps.tile([128, DC, 128], F32, name="ps_ep", tag="s", bufs=3)
                    for dc in range(DC):
                        nc.tensor.transpose(tp[:, dc, :],
                                            cacc[:, dc, qt * 128:(qt + 1) * 128], ident)
                    tpf = tp.rearrange("p a b -> p (a b)")
                    if base_sbuf is None:
                        bt = pool.tile([128, D], F32, name="ep_b", tag="ln_x", bufs=2)
                        nc.sync.dma_start(out=bt, in_=base_dram[qt].bitcast(F32))
                        nc.vector.tensor_add(out=xout[:, qt, :], in0=tpf, in1=bt)
                    else:
                        nc.vector.tensor_add(out=xout[:, qt, :], in0=tpf,
                                             in1=base_sbuf[:, qt, :])

            # ================= pass =================
            ln_to_T(dec_full, LT, y1T, "ln1_gc", "ln1_bc")
            ln_to_T(dec_own, QT, y1To, "ln1_gc", "ln1_bc")
            if debug:
                for dc in range(DC):
                    for lt in range(2):
                        dt0 = pool.tile([128, 512], F32, name="dbg0", tag="dbg", bufs=2)
                        nc.vector.tensor_copy(
                            out=dt0, in_=y1T[:, dc, lt * 512:(lt + 1) * 512].bitcast(F32))
                        nc.sync.dma_start(
                            out=dbg["dbg_y1T"][:, dc, lt * 512:(lt + 1) * 512], in_=dt0)

            attn1 = tc.alloc_tile_pool(name="attn1", bufs=1)
            cacc1 = pool.tile([128, DC, LQ], F32, name="cacc1", tag="cacc")
            attention("sa", attn1, y1To, y1T, L, True, cacc1)
            epilogue(cacc1, biasc["sa"], dec_own, None, x1)
            attn1.release()
            sa_pool.release()
            if debug:
                for qt in range(QT):
                    nc.sync.dma_start(out=dbg["dbg_x1"][qt], in_=x1[:, qt, :])

            y2To = pool.tile([128, DC, LQ], F32R, name="y2To", tag="yTo")
            ln_to_T(None, QT, y2To, "ln2_gc", "ln2_bc", x_sbuf=x1)
            attn2 = tc.alloc_tile_pool(name="attn2", bufs=1)
            cacc2 = pool.tile([128, DC, LQ], F32, name="cacc2", tag="cacc")
            attention("ca", attn2, y2To, enc_t, L, False, cacc2)
            epilogue(cacc2, biasc["ca"], None, x1, x2)
            attn2.release()
            if debug:
                for qt in range(QT):
                    nc.sync.dma_start(out=dbg["dbg_x2"][qt], in_=x2[:, qt, :])

            y3To = pool.tile([128, DC, LQ], F32R, name="y3To", tag="yTo")
            ln_to_T(None, QT, y3To, "ln3_gc", "ln3_bc", x_sbuf=x2)

            # ---------- FFN ----------
            ffn = tc.alloc_tile_pool(name="ffn", bufs=1)
            w1_t = ffn.tile([128, DC, F], F32R)
            for dc in range(DC):
                nc.sync.dma_start(out=w1_t[:, dc, :], in_=ff_w1T[dc])
            w2_t = ffn.tile([128, FC, D], F32R)
            for fc in range(FC):
                nc.sync.dma_start(out=w2_t[:, fc, :], in_=ff_w2T[fc])
            uT = ffn.tile([128, FC, LQ], F32R)
            for fc in range(FC):
                pu = ps.tile([128, LQ], F32, name="ps_u", tag="s", bufs=3)
                for dc in range(DC):
                    nc.tensor.matmul(pu, lhsT=w1_t[:, dc, fc * 128:(fc + 1) * 128],
                                     rhs=y3To[:, dc, :], start=dc == 0, stop=dc == DC - 1)
                a = ffn.tile([128, LQ], F32, name="ff_a", tag="ff_a", bufs=2)
                nc.vector.tensor_scalar_add(out=a, in0=pu, scalar1=b1t[:, fc:fc + 1])
                mn = ffn.tile([128, LQ], F32, name="ff_m", tag="ff_m", bufs=2)
                nc.vector.tensor_scalar_min(out=mn, in0=a, scalar1=0.0)
                ex = ffn.tile([128, LQ], F32, name="ff_e", tag="ff_e", bufs=2)
                nc.scalar.activation(out=ex, in_=mn, func=AF.Exp, bias=zero_t)
                p1 = ffn.tile([128, LQ], F32, name="ff_p", tag="ff_p", bufs=2)
                nc.vector.tensor_scalar(out=p1, in0=a, scalar1=0.0, scalar2=-1.0,
                                        op0=OP.max, op1=OP.add)
                nc.vector.tensor_add(out=uT[:, fc, :], in0=ex, in1=p1)
            fT = ffn.tile([128, DC, LQ], F32)
            for dc in range(DC):
                pf = ps.tile([128, LQ], F32, name="ps_f", tag="big", bufs=4)
                for fc in range(FC):
                    nc.tensor.matmul(pf, lhsT=w2_t[:, fc, dc * 128:(dc + 1) * 128],
                                     rhs=uT[:, fc, :], start=fc == 0, stop=fc == FC - 1)
                nc.vector.tensor_scalar_add(out=fT[:, dc, :], in0=pf,
                                            scalar1=b2t[:, dc:dc + 1])
            for qt in range(QT):
                tp = ps.tile([128, DC, 128], F32, name="ps_ft", tag="s", bufs=3)
                for dc in range(DC):
                    nc.tensor.transpose(tp[:, dc, :],
                                        fT[:, dc, qt * 128:(qt + 1) * 128], ident)
                ot = ffn.tile([128, D], F32, name="out_t", tag="out_t", bufs=2)
                nc.vector.tensor_add(out=ot, in0=tp.rearrange("p a b -> p (a b)"),
                                     in1=x2[:, qt, :])
                nc.sync.dma_start(out=out[qt], in_=ot)
            ffn.release()
            pool.release()

        if n_iter > 0:
            with tc.For_i(0, n_iter, 1):
                body()
        else:
            body()

    nc.compile()
    return nc


# --------------------------------------------------------------------------
# host side
# --------------------------------------------------------------------------
def _prep_shared(params):
    P = {k: np.ascontiguousarray(np.asarray(v, dtype=np.float32))
         for k, v in params.items()}
    s = {}
    for p in ("sa", "ca"):
        s[p + "_wqT"] = np.ascontiguousarray(
            P[p + "_Wq"].transpose(0, 2, 1)).reshape(H, DC, 128, E)
        s[p + "_wkT"] = np.ascontiguousarray(
            P[p + "_Wk"].transpose(0, 2, 1)).reshape(H, DC, 128, E)
        s[p + "_wvT"] = np.ascontiguousarray(
            P[p + "_Wv"].transpose(0, 2, 1)).reshape(H, DC, 128, E)
        s[p + "_wcT"] = np.ascontiguousarray(P[p + "_Wc"].T).reshape(WCC, 128, D)
        s[p + "_bqc"] = np.ascontiguousarray(
            P[p + "_bq"].reshape(H, ECH, 128).transpose(2, 0, 1))
        s[p + "_bkc"] = np.ascontiguousarray(
            P[p + "_bk"].reshape(H, ECH, 128).transpose(2, 0, 1))
        s[p + "_bv"] = P[p + "_bv"]
        s[p + "_bcc"] = np.ascontiguousarray(P[p + "_bc"].reshape(DC, 128).T)
    s["ff_w1T"] = np.ascontiguousarray(P["ff_W1"].T).reshape(DC, 128, F)
    s["ff_w2T"] = np.ascontiguousarray(P["ff_W2"].T).reshape(FC, 128, D)
    s["ff_b1c"] = np.ascontiguousarray(P["ff_b1"].reshape(FC, 128).T)
    s["ff_b2c"] = np.ascontiguousarray(P["ff_b2"].reshape(DC, 128).T)
    for i in (1, 2, 3):
        s[f"ln{i}_gc"] = np.ascontiguousarray(P[f"ln{i}_g"].reshape(DC, 128).T)
        s[f"ln{i}_bc"] = np.ascontiguousarray(P[f"ln{i}_b"].reshape(DC, 128).T)
    return s


def _prep_percore(decoder_embeddings, encoder_embeddings):
    dec = np.asarray(decoder_embeddings, dtype=np.float32)
    enc = np.asarray(encoder_embeddings, dtype=np.float32)
    per = []
    k_global = (np.arange(128)[:, None] + 128 * np.arange(LT)[None, :])  # [128, LT]
    for c in range(NCORES):
        b, half = c // 2, c % 2
        q0 = half * LQ
        m = (k_global[:, :, None] <= (q0 + np.arange(LQ))[None, None, :])
        per.append({
            "dec_full": np.ascontiguousarray(dec[b].reshape(LT, 128, D)),
            "dec_own": np.ascontiguousarray(dec[b, q0:q0 + LQ].reshape(QT, 128, D)),
            "encT": np.ascontiguousarray(enc[b].T).reshape(DC, 128, L),
            "mask": np.ascontiguousarray(m.astype(np.float32)),
        })
    return per


def make_runner(nc):
    """Persistent jitted SPMD runner. Weights are replicated (no 8x concat)."""
    import jax
    from jax.sharding import Mesh, PartitionSpec
    from jax.experimental.shard_map import shard_map
    from concourse import mybir
    from concourse.bass2jax import (_bass_exec_p, install_neuronx_cc_hook,
                                    partition_id_tensor)
    install_neuronx_cc_hook()

    in_names, out_names, out_avals = [], [], []
    for alloc in nc.m.functions[0].allocations:
        if not isinstance(alloc, mybir.MemoryLocationSet):
            continue
        name = alloc.memorylocations[0].name
        if alloc.kind == "ExternalInput":
            in_names.append(name)
        elif alloc.kind == "ExternalOutput":
            out_names.append(name)
            out_avals.append(jax.core.ShapedArray(tuple(alloc.tensor_shape),
                                                  mybir.dt.np(alloc.dtype)))
    pname = nc.partition_id_tensor.name if nc.partition_id_tensor else None
    if pname:
        in_names.remove(pname)
    all_in = in_names + out_names + ([pname] if pname else [])

    def _body(*args):
        ops = list(args)
        if pname:
            ops.append(partition_id_tensor())
        return tuple(_bass_exec_p.bind(
            *ops, out_avals=tuple(out_avals), in_names=tuple(all_in),
            out_names=tuple(out_names), lowering_input_output_aliases=(),
            sim_require_finite=True, sim_require_nnan=True, nc=nc))

    mesh = Mesh(np.asarray(jax.devices()[:NCORES]), ("core",))
    in_specs = tuple(PartitionSpec("core") if n in PER_CORE_INPUTS else PartitionSpec()
                     for n in in_names) + (PartitionSpec("core"),) * len(out_names)
    out_specs = (PartitionSpec("core"),) * len(out_names)
    f = jax.jit(shard_map(_body, mesh=mesh, in_specs=in_specs, out_specs=out_specs,
                          check_rep=False), keep_unused=True)

    def run(shared, per_core, n_rep=1):
        import time
        args = []
        for n in in_names:
            if n in PER_CORE_INPUTS:
                args.append(np.concatenate([per_core[c][n] for c in range(NCORES)],
                                           axis=0))
            else:
                args.append(shared[n])
        for a in out_avals:
            args.append(np.zeros((NCORES * a.shape[0], *a.shape[1:]), a.dtype))
        args = [jax.device_put(a) for a in args]
        outs = f(*args)
        jax.block_until_ready(outs)
        walls = []
        for _ in range(max(0, n_rep - 1)):
            t0 = time.time()
            outs = f(*args)
            jax.block_until_ready(outs)
            walls.append(time.time() - t0)
        res = {nm: np.asarray(outs[i]).reshape(NCORES, *out_avals[i].shape)
               for i, nm in enumerate(out_names)}
        return res, walls

    return run


def _get(key, builder):
    if key not in _CACHE:
        _CACHE[key] = builder()
    return _CACHE[key]


def kernel(decoder_embeddings, encoder_embeddings, params):
    runner = _get("runner0", lambda: make_runner(_get("nc0", lambda: build_nc())))
    shared = _prep_shared(params)
    per_core = _prep_percore(decoder_embeddings, encoder_embeddings)
    res, _ = runner(shared, per_core)
    outs = res["out"]  # [8, QT, 128, D]
    full = np.empty((B, L, D), dtype=np.float32)
    for c in range(NCORES):
        b, half = c // 2, c % 2
        full[b, half * LQ:(half + 1) * LQ] = outs[c].reshape(LQ, D)
    return full


# revision 19
# speedup vs baseline: 1.0581x; 1.0575x over previous
"""Trainium2 Bass kernel for nn_DecoderLayer (dense transformer decoder layer).

Sharding: 8 cores = 4 batches x 2 query-halves (512 query rows each).
Uniform SPMD program across cores; causality enters via per-core 0/1 mask
*data* (multiplied after exp), not program structure.

All matmuls run as fp32r (fp32 bits on the full-rate PE path) with N=512.
Attention uses transposed scores S^T[k,q] so no P-transpose is ever needed:
  y^T = LN(x)^T (PE transpose),  Q^T/K^T = W^T-stationary projections,
  S^T[k,q] = K^T-chunk-stationary @ Q^T,  P^T = exp(S^T*scale)*mask,
  sums = ones-column matmul,  O^T[e,q] = V-stationary @ P^T (accumulated raw),
  normalize O^T by PE-outer-product broadcast of 1/sums,
  C^T[d,q] = Wc^T-stationary @ O^T accumulated over heads.
Residual stream kept in N layout [tokens, d]; PE transposes convert.

Self-contained: imports only /opt/trn_rl_repo + numpy/jax.
"""
import sys
sys.path.insert(0, '/opt/trn_rl_repo')

import math
import numpy as np

B, L, D, H, E, F = 4, 1024, 512, 8, 512, 2048
EPS = 1e-5
SCALE = 1.0 / math.sqrt(D)
LQ = 512
NCORES = 8
DC = D // 128
ECH = E // 128
LT = L // 128
QT = LQ // 128
FC = F // 128
WCC = (H * E) // 128

PER_CORE_INPUTS = ("dec_full", "dec_own", "encT", "mask")

_CACHE = {}


# --------------------------------------------------------------------------
# device program
# --------------------------------------------------------------------------
def build_nc(debug=False, n_iter=0, fake_share_weights=False, trace_sim=False):
    from contextlib import ExitStack
    from concourse import bacc, mybir, tile
    from concourse.masks import make_identity

    F32 = mybir.dt.float32
    F32R = mybir.dt.float32r
    AF = mybir.ActivationFunctionType
    OP = mybir.AluOpType

    nc = bacc.Bacc("TRN2", target_bir_lowering=False, debug=False)

    def din(name, shape, dt=F32R):
        return nc.dram_tensor(name, shape, dt, kind="ExternalInput").ap()

    dec_full = din("dec_full", [LT, 128, D])
    dec_own = din("dec_own", [QT, 128, D])
    encT_d = din("encT", [DC, 128, L])
    mask_d = din("mask", [128, LT, LQ])
    w_in = {}
    for p in ("sa", "ca"):
        w_in[p + "_wqT"] = din(p + "_wqT", [H, DC, 128, E])
        w_in[p + "_wkT"] = din(p + "_wkT", [H, DC, 128, E])
        w_in[p + "_wvT"] = din(p + "_wvT", [H, DC, 128, E])
        w_in[p + "_wcT"] = din(p + "_wcT", [WCC, 128, D])
        w_in[p + "_bqc"] = din(p + "_bqc", [128, H, ECH], F32)
        w_in[p + "_bkc"] = din(p + "_bkc", [128, H, ECH], F32)
        w_in[p + "_bv"] = din(p + "_bv", [H, E])
        w_in[p + "_bcc"] = din(p + "_bcc", [128, DC], F32)
    ff_w1T = din("ff_w1T", [DC, 128, F])
    ff_w2T = din("ff_w2T", [FC, 128, D])
    ff_b1c = din("ff_b1c", [128, FC], F32)
    ff_b2c = din("ff_b2c", [128, DC], F32)
    lnc = {k: din(k, [128, DC], F32) for k in
           ("ln1_gc", "ln1_bc", "ln2_gc", "ln2_bc", "ln3_gc", "ln3_bc")}

    out = nc.dram_tensor("out", [QT, 128, D], F32, kind="ExternalOutput").ap()
    dbg = {}
    if debug:
        for nm, shp in (("dbg_y1T", [128, DC, L]), ("dbg_kT", [128, ECH, L]),
                        ("dbg_pT", [128, LT, LQ]), ("dbg_oT", [128, ECH, LQ]),
                        ("dbg_x1", [QT, 128, D]), ("dbg_x2", [QT, 128, D])):
            dbg[nm] = nc.dram_tensor(nm, shp, F32, kind="ExternalOutput").ap()

    with ExitStack() as ctx:
        ctx.enter_context(nc.allow_low_precision(reason="float32r has float32 bits"))
        tc = ctx.enter_context(tile.TileContext(nc, trace_sim=trace_sim))
        per = ctx.enter_context(tc.tile_pool(name="per", bufs=1))
        ps = ctx.enter_context(tc.tile_pool(name="ps", bufs=1, space="PSUM"))

        ident = per.tile([128, 128], F32)
        make_identity(nc, ident)
        ones_f32 = per.tile([128, 1], F32)
        nc.vector.memset(ones_f32, 1.0)
        ones_k = per.tile([128, 1], F32R)
        nc.vector.tensor_copy(out=ones_k, in_=ones_f32)
        ones_r = per.tile([1, 128], F32R)
        nc.vector.tensor_copy(out=ones_r, in_=ones_f32[0:1, 0:1].to_broadcast((1, 128)))
        eps_t = per.tile([128, 1], F32)
        nc.vector.memset(eps_t, EPS)
        zero_t = per.tile([128, 1], F32)
        nc.vector.memset(zero_t, 0.0)

        def load_small(ap_in, shape, nm):
            t = per.tile(shape, F32, name=nm)
            nc.sync.dma_start(out=t, in_=ap_in)
            return t

        biasq = {p: load_small(w_in[p + "_bqc"], [128, H, ECH], p + "bq") for p in ("sa", "ca")}
        biask = {p: load_small(w_in[p + "_bkc"], [128, H, ECH], p + "bk") for p in ("sa", "ca")}
        biasc = {p: load_small(w_in[p + "_bcc"], [128, DC], p + "bc") for p in ("sa", "ca")}
        lng = {k: load_small(v, [128, DC], k) for k, v in lnc.items()}
        b1t = load_small(ff_b1c, [128, FC], "b1t")
        b2t = load_small(ff_b2c, [128, DC], "b2t")

        enc_t = per.tile([128, DC, L], F32R)
        for dc in range(DC):
            nc.sync.dma_start(out=enc_t[:, dc, :], in_=encT_d[dc])

        def body():
            pool = tc.alloc_tile_pool(name="work", bufs=1)
            sa_pool = tc.alloc_tile_pool(name="sa_only", bufs=1)

            mask_t = sa_pool.tile([128, LT, LQ], F32R)
            for kt in range(LT):
                nc.sync.dma_start(out=mask_t[:, kt, :], in_=mask_d[:, kt, :])
            y1T = sa_pool.tile([128, DC, L], F32R)

            y1To = pool.tile([128, DC, LQ], F32R, tag="yTo")
            x1 = pool.tile([128, QT, D], F32, tag="x1")
            x2 = pool.tile([128, QT, D], F32, tag="x2")

            # ---------- LayerNorm -> transposed output ----------
            def ln_to_T(src_dram, n_tiles, yT_dst, gk, bk, x_sbuf=None):
                for it in range(n_tiles):
                    if x_sbuf is None:
                        xt = pool.tile([128, D], F32, name="ln_x", tag="ln_x", bufs=2)
                        nc.sync.dma_start(out=xt, in_=src_dram[it].bitcast(F32))
                    else:
                        xt = x_sbuf[:, it, :]
                    stats = pool.tile([128, 6], F32, name="ln_s", tag="ln_s", bufs=2)
                    nc.vector.bn_stats(out=stats, in_=xt)
                    mv = pool.tile([128, 2], F32, name="ln_mv", tag="ln_mv", bufs=2)
                    nc.vector.bn_aggr(out=mv, in_=stats)
                    std = pool.tile([128, 1], F32, name="ln_d", tag="ln_d", bufs=2)
                    nc.scalar.activation(out=std, in_=mv[:, 1:2], func=AF.Sqrt,
                                         bias=eps_t, scale=1.0)
                    rstd = pool.tile([128, 1], F32, name="ln_r", tag="ln_r", bufs=2)
                    nc.vector.reciprocal(out=rstd, in_=std)
                    yn = pool.tile([128, D], F32, name="ln_y", tag="ln_y", bufs=2)
                    nc.vector.tensor_scalar(out=yn, in0=xt, scalar1=mv[:, 0:1],
                                            scalar2=rstd, op0=OP.subtract, op1=OP.mult)
                    tp = ps.tile([128, DC, 128], F32, name="ps_tp", tag="s", bufs=3)
                    for dc in range(DC):
                        nc.tensor.transpose(tp[:, dc, :],
                                            yn[:, dc * 128:(dc + 1) * 128], ident)
                    for dc in range(DC):
                        nc.vector.tensor_scalar(
                            out=yT_dst[:, dc, it * 128:(it + 1) * 128],
                            in0=tp[:, dc, :], scalar1=lng[gk][:, dc:dc + 1],
                            scalar2=lng[bk][:, dc:dc + 1], op0=OP.mult, op1=OP.add)

            # ---------- attention ----------
            def attention(prefix, pool_a, qT_own, kvT, kv_len, use_mask, cacc):
                nkt = kv_len // 128

                def emit_combine(oTt, wct, first):
                    # C^T partial for a finished head; PSUM from "s" tag so the
                    # next head's AV accumulators ("big") are never blocked.
                    for dc in range(DC):
                        pc = ps.tile([128, LQ], F32, name="ps_c", tag="s", bufs=3)
                        for ec in range(ECH):
                            nc.tensor.matmul(pc,
                                             lhsT=wct[:, ec, dc * 128:(dc + 1) * 128],
                                             rhs=oTt[:, ec, :],
                                             start=ec == 0, stop=ec == ECH - 1)
                        if first:
                            nc.vector.tensor_copy(out=cacc[:, dc, :], in_=pc)
                        else:
                            nc.vector.tensor_add(out=cacc[:, dc, :],
                                                 in0=cacc[:, dc, :], in1=pc)

                pend = None
                for h in range(H):
                    wq_t = pool_a.tile([128, DC, E], F32R, name="wq", tag="wq", bufs=2)
                    wk_t = pool_a.tile([128, DC, E], F32R, name="wk", tag="wk", bufs=1)
                    wv_t = pool_a.tile([128, DC, E], F32R, name="wv", tag="wv", bufs=1)
                    for dc in range(DC):
                        nc.sync.dma_start(out=wq_t[:, dc, :], in_=w_in[prefix + "_wqT"][h, dc])
                        nc.sync.dma_start(out=wk_t[:, dc, :], in_=w_in[prefix + "_wkT"][h, dc])
                        nc.sync.dma_start(out=wv_t[:, dc, :], in_=w_in[prefix + "_wvT"][h, dc])
                    wc_t = pool_a.tile([128, ECH, D], F32R, name="wc", tag="wc", bufs=2)
                    for ec in range(ECH):
                        nc.sync.dma_start(out=wc_t[:, ec, :],
                                          in_=w_in[prefix + "_wcT"][h * ECH + ec])
                    bv_t = pool_a.tile([128, E], F32R, name="bv", tag="bv", bufs=2)
                    nc.sync.dma_start(
                        out=bv_t,
                        in_=w_in[prefix + "_bv"][h:h + 1, :].to_broadcast((128, E)))

                    qT = pool_a.tile([128, ECH, LQ], F32R, name="qT", tag="qT", bufs=1)
                    kT = pool_a.tile([128, ECH, kv_len], F32R, name="kT", tag="kT", bufs=1)
                    for ec in range(ECH):
                        for lt in range(kv_len // 512):
                            acck = ps.tile([128, 512], F32, name="ps_k", tag="s", bufs=3)
                            for dc in range(DC):
                                nc.tensor.matmul(
                                    acck, lhsT=wk_t[:, dc, ec * 128:(ec + 1) * 128],
                                    rhs=kvT[:, dc, lt * 512:(lt + 1) * 512],
                                    start=dc == 0, stop=dc == DC - 1)
                            nc.scalar.activation(
                                out=kT[:, ec, lt * 512:(lt + 1) * 512], in_=acck,
                                func=AF.Identity,
                                bias=biask[prefix][:, h, ec:ec + 1], scale=1.0)
                    for ec in range(ECH):
                        accq = ps.tile([128, LQ], F32, name="ps_q", tag="s", bufs=3)
                        for dc in range(DC):
                            nc.tensor.matmul(accq,
                                             lhsT=wq_t[:, dc, ec * 128:(ec + 1) * 128],
                                             rhs=qT_own[:, dc, :],
                                             start=dc == 0, stop=dc == DC - 1)
                        nc.scalar.activation(
                            out=qT[:, ec, :], in_=accq, func=AF.Identity,
                            bias=biasq[prefix][:, h, ec:ec + 1], scale=1.0)

                    # streaming over key tiles: V jit, scores, exp, mask, sums, AV
                    sums = ps.tile([1, LQ], F32, name="ps_sum", tag="sum", bufs=1)
                    po = [ps.tile([128, LQ], F32, name=f"ps_o{ec}", tag="big", bufs=4)
                          for ec in range(ECH)]
                    for kt in range(nkt):
                        accv = ps.tile([128, E], F32, name="ps_v", tag="s", bufs=3)
                        for dc in range(DC):
                            nc.tensor.matmul(accv,
                                             lhsT=kvT[:, dc, kt * 128:(kt + 1) * 128],
                                             rhs=wv_t[:, dc, :],
                                             start=dc == 0, stop=dc == DC - 1)
                        vt = pool_a.tile([128, E], F32R, name="vt", tag="vt", bufs=2)
                        nc.vector.tensor_add(out=vt, in0=accv, in1=bv_t)

                        sc = ps.tile([128, LQ], F32, name="ps_s", tag="s", bufs=3)
                        for ec in range(ECH):
                            nc.tensor.matmul(sc,
                                             lhsT=kT[:, ec, kt * 128:(kt + 1) * 128],
                                             rhs=qT[:, ec, :],
                                             start=ec == 0, stop=ec == ECH - 1)
                        pt = pool_a.tile([128, LQ], F32R, name="pt", tag="pt", bufs=2)
                        nc.scalar.activation(out=pt, in_=sc, func=AF.Exp, scale=SCALE,
                                             bias=zero_t)
                        if use_mask:
                            nc.gpsimd.tensor_mul(out=pt, in0=pt, in1=mask_t[:, kt, :])
                        nc.tensor.matmul(sums, lhsT=ones_k, rhs=pt,
                                         start=kt == 0, stop=kt == nkt - 1)
                        for ec in range(ECH):
                            nc.tensor.matmul(po[ec],
                                             lhsT=vt[:, ec * 128:(ec + 1) * 128],
                                             rhs=pt, start=kt == 0, stop=kt == nkt - 1)
                        if debug and prefix == "sa" and h == 0:
                            dt1 = pool.tile([128, LQ], F32, name="dbg1", tag="dbg", bufs=2)
                            nc.vector.tensor_copy(out=dt1, in_=pt.bitcast(F32))
                            nc.sync.dma_start(out=dbg["dbg_pT"][:, kt, :], in_=dt1)

                    # previous head's combine fills the PE while this head's
                    # softmax-tail chain (DVE/ACT latency) resolves
                    if pend is not None:
                        emit_combine(*pend)

                    recip = pool_a.tile([1, LQ], F32R, name="recip", tag="recip", bufs=2)
                    nc.vector.reciprocal(out=recip, in_=sums)
                    rb_ps = ps.tile([128, LQ], F32, name="ps_rb", tag="s", bufs=3)
                    nc.tensor.matmul(rb_ps, lhsT=ones_r, rhs=recip, start=True, stop=True)
                    rb = pool_a.tile([128, LQ], F32, name="rb", tag="rb", bufs=2)
                    nc.vector.tensor_copy(out=rb, in_=rb_ps)

                    oT = pool_a.tile([128, ECH, LQ], F32R, name="oT", tag="oT", bufs=2)
                    for ec in range(ECH):
                        nc.vector.tensor_mul(out=oT[:, ec, :], in0=po[ec], in1=rb)
                    if debug and prefix == "sa" and h == 0:
                        for ec in range(ECH):
                            dt2 = pool.tile([128, LQ], F32, name="dbg2", tag="dbg", bufs=2)
                            nc.vector.tensor_copy(out=dt2, in_=oT[:, ec, :].bitcast(F32))
                            nc.sync.dma_start(out=dbg["dbg_oT"][:, ec, :], in_=dt2)
                    pend = (oT, wc_t, h == 0)
                emit_combine(*pend)

            # ---------- epilogue: xout(N) = base(N) + transpose(cacc + bc) ----------
            def epilogue(cacc, bcc_t, base_dram, base_sbuf, xout):
                for dc in range(DC):
                    nc.vector.tensor_scalar_add(out=cacc[:, dc, :], in0=cacc[:, dc, :],
                                                scalar1=bcc_t[:, dc:dc + 1])
                for qt in range(QT):
                    tp = ps.tile([128, DC, 128], F32, name="ps_ep", tag="s", bufs=3)
                    for dc in range(DC):
                        nc.tensor.transpose(tp[:, dc, :],
                                            cacc[:, dc, qt * 128:(qt + 1) * 128], ident)
                    tpf = tp.rearrange("p a b -> p (a b)")
                    if base_sbuf is None:
                        bt = pool.tile([128, D], F32, name="ep_b", tag="ln_x", bufs=2)
                        nc.sync.dma_start(out=bt, in_=base_dram[qt].bitcast(F32))
                        nc.vector.tensor_add(out=xout[:, qt, :], in0=tpf, in1=bt)
                    else:
                        nc.vector.tensor_add(out=xout[:, qt, :], in0=tpf,
                                             in1=base_sbuf[:, qt, :])

            # ================= pass =================
            ln_to_T(dec_full, LT, y1T, "ln1_gc", "ln1_bc")
            ln_to_T(dec_own, QT, y1To, "ln1_gc", "ln1_bc")
            if debug:
                for dc in range(DC):
                    for lt in range(2):
                        dt0 = pool.tile([128, 512], F32, name="dbg0", tag="dbg", bufs=2)
                        nc.vector.tensor_copy(
                            out=dt0, in_=y1T[:, dc, lt * 512:(lt + 1) * 512].bitcast(F32))
                        nc.sync.dma_start(
                            out=dbg["dbg_y1T"][:, dc, lt * 512:(lt + 1) * 512], in_=dt0)

            attn1 = tc.alloc_tile_pool(name="attn1", bufs=1)
            cacc1 = pool.tile([128, DC, LQ], F32, name="cacc1", tag="cacc")
            attention("sa", attn1, y1To, y1T, L, True, cacc1)
            epilogue(cacc1, biasc["sa"], dec_own, None, x1)
            attn1.release()
            sa_pool.release()
            ffn_w1a = tc.alloc_tile_pool(name="ffn_w1a", bufs=1)
            w1a_t = ffn_w1a.tile([128, DC, F // 2], F32R)
            for dc in range(DC):
                nc.sync.dma_start(out=w1a_t[:, dc, :], in_=ff_w1T[dc, :, :F // 2])
            if debug:
                for qt in range(QT):
                    nc.sync.dma_start(out=dbg["dbg_x1"][qt], in_=x1[:, qt, :])

            y2To = pool.tile([128, DC, LQ], F32R, name="y2To", tag="yTo")
            ln_to_T(None, QT, y2To, "ln2_gc", "ln2_bc", x_sbuf=x1)
            attn2 = tc.alloc_tile_pool(name="attn2", bufs=1)
            cacc2 = pool.tile([128, DC, LQ], F32, name="cacc2", tag="cacc")
            attention("ca", attn2, y2To, enc_t, L, False, cacc2)
            epilogue(cacc2, biasc["ca"], None, x1, x2)
            attn2.release()
            if debug:
                for qt in range(QT):
                    nc.sync.dma_start(out=dbg["dbg_x2"][qt], in_=x2[:, qt, :])

            y3To = pool.tile([128, DC, LQ], F32R, name="y3To", tag="yTo")
            ln_to_T(None, QT, y3To, "ln3_gc", "ln3_bc", x_sbuf=x2)

            # ---------- FFN ----------
            ffn = tc.alloc_tile_pool(name="ffn", bufs=1)
            w1b_t = ffn.tile([128, DC, F // 2], F32R)
            for dc in range(DC):
                nc.sync.dma_start(out=w1b_t[:, dc, :], in_=ff_w1T[dc, :, F // 2:])
            w2_t = ffn.tile([128, FC, D], F32R)
            for fc in range(FC):
                nc.sync.dma_start(out=w2_t[:, fc, :], in_=ff_w2T[fc])
            uT = ffn.tile([128, FC, LQ], F32R)
            for fc in range(FC):
                pu = ps.tile([128, LQ], F32, name="ps_u", tag="s", bufs=3)
                wsrc = w1a_t if fc < FC // 2 else w1b_t
                fcl = fc if fc < FC // 2 else fc - FC // 2
                for dc in range(DC):
                    nc.tensor.matmul(pu, lhsT=wsrc[:, dc, fcl * 128:(fcl + 1) * 128],
                                     rhs=y3To[:, dc, :], start=dc == 0, stop=dc == DC - 1)
                a = ffn.tile([128, LQ], F32, name="ff_a", tag="ff_a", bufs=2)
                nc.scalar.activation(out=a, in_=pu, func=AF.Identity,
                                     bias=b1t[:, fc:fc + 1], scale=1.0)
                mn = ffn.tile([128, LQ], F32, name="ff_m", tag="ff_m", bufs=2)
                nc.vector.tensor_scalar_min(out=mn, in0=a, scalar1=0.0)
                ex = ffn.tile([128, LQ], F32, name="ff_e", tag="ff_e", bufs=2)
                nc.scalar.activation(out=ex, in_=mn, func=AF.Exp, bias=zero_t)
                p1 = ffn.tile([128, LQ], F32, name="ff_p", tag="ff_p", bufs=2)
                nc.vector.tensor_scalar(out=p1, in0=a, scalar1=0.0, scalar2=-1.0,
                                        op0=OP.max, op1=OP.add)
                nc.vector.tensor_add(out=uT[:, fc, :], in0=ex, in1=p1)
            fT = ffn.tile([128, DC, LQ], F32)
            for dc in range(DC):
                pf = ps.tile([128, LQ], F32, name="ps_f", tag="big", bufs=4)
                for fc in range(FC):
                    nc.tensor.matmul(pf, lhsT=w2_t[:, fc, dc * 128:(dc + 1) * 128],
                                     rhs=uT[:, fc, :], start=fc == 0, stop=fc == FC - 1)
                nc.scalar.activation(out=fT[:, dc, :], in_=pf, func=AF.Identity,
                                     bias=b2t[:, dc:dc + 1], scale=1.0)
            for qt in range(QT):
                tp = ps.tile([128, DC, 128], F32, name="ps_ft", tag="s", bufs=3)
                for dc in range(DC):
                    nc.tensor.transpose(tp[:, dc, :],
                                        fT[:, dc, qt * 128:(qt + 1) * 128], ident)
                ot = ffn.tile([128, D], F32, name="out_t", tag="out_t", bufs=2)
                nc.vector.tensor_add(out=ot, in0=tp.rearrange("p a b -> p (a b)"),
                                     in1=x2[:, qt, :])
                nc.sync.dma_start(out=out[qt], in_=ot)
            ffn.release()
            ffn_w1a.release()
            pool.release()

        if n_iter > 0:
            with tc.For_i(0, n_iter, 1):
                body()
        else:
            body()

    nc.compile()
    return nc


# --------------------------------------------------------------------------
# host side
# --------------------------------------------------------------------------
def _prep_shared(params):
    P = {k: np.ascontiguousarray(np.asarray(v, dtype=np.float32))
         for k, v in params.items()}
    s = {}
    for p in ("sa", "ca"):
        s[p + "_wqT"] = np.ascontiguousarray(
            P[p + "_Wq"].transpose(0, 2, 1)).reshape(H, DC, 128, E)
        s[p + "_wkT"] = np.ascontiguousarray(
            P[p + "_Wk"].transpose(0, 2, 1)).reshape(H, DC, 128, E)
        s[p + "_wvT"] = np.ascontiguousarray(
            P[p + "_Wv"].transpose(0, 2, 1)).reshape(H, DC, 128, E)
        s[p + "_wcT"] = np.ascontiguousarray(P[p + "_Wc"].T).reshape(WCC, 128, D)
        s[p + "_bqc"] = np.ascontiguousarray(
            P[p + "_bq"].reshape(H, ECH, 128).transpose(2, 0, 1))
        s[p + "_bkc"] = np.ascontiguousarray(
            P[p + "_bk"].reshape(H, ECH, 128).transpose(2, 0, 1))
        s[p + "_bv"] = P[p + "_bv"]
        s[p + "_bcc"] = np.ascontiguousarray(P[p + "_bc"].reshape(DC, 128).T)
    s["ff_w1T"] = np.ascontiguousarray(P["ff_W1"].T).reshape(DC, 128, F)
    s["ff_w2T"] = np.ascontiguousarray(P["ff_W2"].T).reshape(FC, 128, D)
    s["ff_b1c"] = np.ascontiguousarray(P["ff_b1"].reshape(FC, 128).T)
    s["ff_b2c"] = np.ascontiguousarray(P["ff_b2"].reshape(DC, 128).T)
    for i in (1, 2, 3):
        s[f"ln{i}_gc"] = np.ascontiguousarray(P[f"ln{i}_g"].reshape(DC, 128).T)
        s[f"ln{i}_bc"] = np.ascontiguousarray(P[f"ln{i}_b"].reshape(DC, 128).T)
    return s


def _prep_percore(decoder_embeddings, encoder_embeddings):
    dec = np.asarray(decoder_embeddings, dtype=np.float32)
    enc = np.asarray(encoder_embeddings, dtype=np.float32)
    per = []
    k_global = (np.arange(128)[:, None] + 128 * np.arange(LT)[None, :])  # [128, LT]
    for c in range(NCORES):
        b, half = c // 2, c % 2
        q0 = half * LQ
        m = (k_global[:, :, None] <= (q0 + np.arange(LQ))[None, None, :])
        per.append({
            "dec_full": np.ascontiguousarray(dec[b].reshape(LT, 128, D)),
            "dec_own": np.ascontiguousarray(dec[b, q0:q0 + LQ].reshape(QT, 128, D)),
            "encT": np.ascontiguousarray(enc[b].T).reshape(DC, 128, L),
            "mask": np.ascontiguousarray(m.astype(np.float32)),
        })
    return per


def make_runner(nc):
    """Persistent jitted SPMD runner. Weights are replicated (no 8x concat)."""
    import jax
    from jax.sharding import Mesh, PartitionSpec
    from jax.experimental.shard_map import shard_map
    from concourse import mybir
    from concourse.bass2jax import (_bass_exec_p, install_neuronx_cc_hook,
                                    partition_id_tensor)
    install_neuronx_cc_hook()

    in_names, out_names, out_avals = [], [], []
    for alloc in nc.m.functions[0].allocations:
        if not isinstance(alloc, mybir.MemoryLocationSet):
            continue
        name = alloc.memorylocations[0].name
        if alloc.kind == "ExternalInput":
            in_names.append(name)
        elif alloc.kind == "ExternalOutput":
            out_names.append(name)
            out_avals.append(jax.core.ShapedArray(tuple(alloc.tensor_shape),
                                                  mybir.dt.np(alloc.dtype)))
    pname = nc.partition_id_tensor.name if nc.partition_id_tensor else None
    if pname:
        in_names.remove(pname)
    all_in = in_names + out_names + ([pname] if pname else [])

    def _body(*args):
        ops = list(args)
        if pname:
            ops.append(partition_id_tensor())
        return tuple(_bass_exec_p.bind(
            *ops, out_avals=tuple(out_avals), in_names=tuple(all_in),
            out_names=tuple(out_names), lowering_input_output_aliases=(),
            sim_require_finite=True, sim_require_nnan=True, nc=nc))

    mesh = Mesh(np.asarray(jax.devices()[:NCORES]), ("core",))
    in_specs = tuple(PartitionSpec("core") if n in PER_CORE_INPUTS else PartitionSpec()
                     for n in in_names) + (PartitionSpec("core"),) * len(out_names)
    out_specs = (PartitionSpec("core"),) * len(out_names)
    f = jax.jit(shard_map(_body, mesh=mesh, in_specs=in_specs, out_specs=out_specs,
                          check_rep=False), keep_unused=True)

    def run(shared, per_core, n_rep=1):
        import time
        args = []
        for n in in_names:
            if n in PER_CORE_INPUTS:
                args.append(np.concatenate([per_core[c][n] for c in range(NCORES)],
                                           axis=0))
            else:
                args.append(shared[n])
        for a in out_avals:
            args.append(np.zeros((NCORES * a.shape[0], *a.shape[1:]), a.dtype))
        args = [jax.device_put(a) for a in args]
        outs = f(*args)
        jax.block_until_ready(outs)
        walls = []
        for _ in range(max(0, n_rep - 1)):
            t0 = time.time()
            outs = f(*args)
            jax.block_until_ready(outs)
            walls.append(time.time() - t0)
        res = {nm: np.asarray(outs[i]).reshape(NCORES, *out_avals[i].shape)
               for i, nm in enumerate(out_names)}
        return res, walls

    return run


def _get(key, builder):
    if key not in _CACHE:
        _CACHE[key] = builder()
    return _CACHE[key]


def kernel(decoder_embeddings, encoder_embeddings, params):
    runner = _get("runner0", lambda: make_runner(_get("nc0", lambda: build_nc())))
    shared = _prep_shared(params)
    per_core = _prep_percore(decoder_embeddings, encoder_embeddings)
    res, _ = runner(shared, per_core)
    outs = res["out"]  # [8, QT, 128, D]
    full = np.empty((B, L, D), dtype=np.float32)
    for c in range(NCORES):
        b, half = c // 2, c % 2
        full[b, half * LQ:(half + 1) * LQ] = outs[c].reshape(LQ, D)
    return full


# revision 21
# speedup vs baseline: 1.0586x; 1.0004x over previous
"""Trainium2 Bass kernel for nn_DecoderLayer (dense transformer decoder layer).

Sharding: 8 cores = 4 batches x 2 query-halves (512 query rows each).
Uniform SPMD program across cores; causality enters via per-core 0/1 mask
*data* (multiplied after exp), not program structure.

All matmuls run as fp32r (fp32 bits on the full-rate PE path) with N=512.
Attention uses transposed scores S^T[k,q] so no P-transpose is ever needed:
  y^T = LN(x)^T (PE transpose),  Q^T/K^T = W^T-stationary projections,
  S^T[k,q] = K^T-chunk-stationary @ Q^T,  P^T = exp(S^T*scale)*mask,
  sums = ones-column matmul,  O^T[e,q] = V-stationary @ P^T (accumulated raw),
  normalize O^T by PE-outer-product broadcast of 1/sums,
  C^T[d,q] = Wc^T-stationary @ O^T accumulated over heads.
Residual stream kept in N layout [tokens, d]; PE transposes convert.

Self-contained: imports only /opt/trn_rl_repo + numpy/jax.
"""
import sys
sys.path.insert(0, '/opt/trn_rl_repo')

import math
import numpy as np

B, L, D, H, E, F = 4, 1024, 512, 8, 512, 2048
EPS = 1e-5
SCALE = 1.0 / math.sqrt(D)
LQ = 512
NCORES = 8
DC = D // 128
ECH = E // 128
LT = L // 128
QT = LQ // 128
FC = F // 128
WCC = (H * E) // 128

PER_CORE_INPUTS = ("dec_full", "dec_own", "encT", "mask")

_CACHE = {}


# --------------------------------------------------------------------------
# device program
# --------------------------------------------------------------------------
def build_nc(debug=False, n_iter=0, fake_share_weights=False, trace_sim=False):
    from contextlib import ExitStack
    from concourse import bacc, mybir, tile
    from concourse.masks import make_identity

    F32 = mybir.dt.float32
    F32R = mybir.dt.float32r
    AF = mybir.ActivationFunctionType
    OP = mybir.AluOpType

    nc = bacc.Bacc("TRN2", target_bir_lowering=False, debug=False)

    def din(name, shape, dt=F32R):
        return nc.dram_tensor(name, shape, dt, kind="ExternalInput").ap()

    dec_full = din("dec_full", [LT, 128, D])
    dec_own = din("dec_own", [QT, 128, D])
    encT_d = din("encT", [DC, 128, L])
    mask_d = din("mask", [128, LT, LQ])
    w_in = {}
    for p in ("sa", "ca"):
        w_in[p + "_wqT"] = din(p + "_wqT", [H, DC, 128, E])
        w_in[p + "_wkT"] = din(p + "_wkT", [H, DC, 128, E])
        w_in[p + "_wvT"] = din(p + "_wvT", [H, DC, 128, E])
        w_in[p + "_wcT"] = din(p + "_wcT", [WCC, 128, D])
        w_in[p + "_bqc"] = din(p + "_bqc", [128, H, ECH], F32)
        w_in[p + "_bkc"] = din(p + "_bkc", [128, H, ECH], F32)
        w_in[p + "_bv"] = din(p + "_bv", [H, E])
        w_in[p + "_bcc"] = din(p + "_bcc", [128, DC], F32)
    ff_w1T = din("ff_w1T", [DC, 128, F])
    ff_w2T = din("ff_w2T", [FC, 128, D])
    ff_b1c = din("ff_b1c", [128, FC], F32)
    ff_b2c = din("ff_b2c", [128, DC], F32)
    lnc = {k: din(k, [128, DC], F32) for k in
           ("ln1_gc", "ln1_bc", "ln2_gc", "ln2_bc", "ln3_gc", "ln3_bc")}

    out = nc.dram_tensor("out", [QT, 128, D], F32, kind="ExternalOutput").ap()
    dbg = {}
    if debug:
        for nm, shp in (("dbg_y1T", [128, DC, L]), ("dbg_kT", [128, ECH, L]),
                        ("dbg_pT", [128, LT, LQ]), ("dbg_oT", [128, ECH, LQ]),
                        ("dbg_x1", [QT, 128, D]), ("dbg_x2", [QT, 128, D])):
            dbg[nm] = nc.dram_tensor(nm, shp, F32, kind="ExternalOutput").ap()

    with ExitStack() as ctx:
        ctx.enter_context(nc.allow_low_precision(reason="float32r has float32 bits"))
        tc = ctx.enter_context(tile.TileContext(nc, trace_sim=trace_sim))
        per = ctx.enter_context(tc.tile_pool(name="per", bufs=1))
        ps = ctx.enter_context(tc.tile_pool(name="ps", bufs=1, space="PSUM"))

        ident = per.tile([128, 128], F32)
        make_identity(nc, ident)
        ones_f32 = per.tile([128, 1], F32)
        nc.vector.memset(ones_f32, 1.0)
        ones_k = per.tile([128, 1], F32R)
        nc.vector.tensor_copy(out=ones_k, in_=ones_f32)
        ones_r = per.tile([1, 128], F32R)
        nc.vector.tensor_copy(out=ones_r, in_=ones_f32[0:1, 0:1].to_broadcast((1, 128)))
        eps_t = per.tile([128, 1], F32)
        nc.vector.memset(eps_t, EPS)
        zero_t = per.tile([128, 1], F32)
        nc.vector.memset(zero_t, 0.0)

        def load_small(ap_in, shape, nm):
            t = per.tile(shape, F32, name=nm)
            nc.sync.dma_start(out=t, in_=ap_in)
            return t

        biasq = {p: load_small(w_in[p + "_bqc"], [128, H, ECH], p + "bq") for p in ("sa", "ca")}
        biask = {p: load_small(w_in[p + "_bkc"], [128, H, ECH], p + "bk") for p in ("sa", "ca")}
        biasc = {p: load_small(w_in[p + "_bcc"], [128, DC], p + "bc") for p in ("sa", "ca")}
        lng = {k: load_small(v, [128, DC], k) for k, v in lnc.items()}
        b1t = load_small(ff_b1c, [128, FC], "b1t")
        b2t = load_small(ff_b2c, [128, DC], "b2t")

        enc_t = per.tile([128, DC, L], F32R)
        for dc in range(DC):
            nc.sync.dma_start(out=enc_t[:, dc, :], in_=encT_d[dc])

        def body():
            pool = tc.alloc_tile_pool(name="work", bufs=1)
            sa_pool = tc.alloc_tile_pool(name="sa_only", bufs=1)

            mask_t = sa_pool.tile([128, LT, LQ], F32R)
            for kt in range(LT):
                nc.sync.dma_start(out=mask_t[:, kt, :], in_=mask_d[:, kt, :])
            y1T = sa_pool.tile([128, DC, L], F32R)

            y1To = pool.tile([128, DC, LQ], F32R, tag="yTo")
            x1 = pool.tile([128, QT, D], F32, tag="x1")
            x2 = pool.tile([128, QT, D], F32, tag="x2")

            # ---------- LayerNorm -> transposed output ----------
            def ln_to_T(src_dram, n_tiles, yT_dst, gk, bk, x_sbuf=None):
                for it in range(n_tiles):
                    if x_sbuf is None:
                        xt = pool.tile([128, D], F32, name="ln_x", tag="ln_x", bufs=2)
                        nc.sync.dma_start(out=xt, in_=src_dram[it].bitcast(F32))
                    else:
                        xt = x_sbuf[:, it, :]
                    stats = pool.tile([128, 6], F32, name="ln_s", tag="ln_s", bufs=2)
                    nc.vector.bn_stats(out=stats, in_=xt)
                    mv = pool.tile([128, 2], F32, name="ln_mv", tag="ln_mv", bufs=2)
                    nc.vector.bn_aggr(out=mv, in_=stats)
                    std = pool.tile([128, 1], F32, name="ln_d", tag="ln_d", bufs=2)
                    nc.scalar.activation(out=std, in_=mv[:, 1:2], func=AF.Sqrt,
                                         bias=eps_t, scale=1.0)
                    rstd = pool.tile([128, 1], F32, name="ln_r", tag="ln_r", bufs=2)
                    nc.vector.reciprocal(out=rstd, in_=std)
                    yn = pool.tile([128, D], F32, name="ln_y", tag="ln_y", bufs=2)
                    nc.vector.tensor_scalar(out=yn, in0=xt, scalar1=mv[:, 0:1],
                                            scalar2=rstd, op0=OP.subtract, op1=OP.mult)
                    tp = ps.tile([128, DC, 128], F32, name="ps_tp", tag="s", bufs=3)
                    for dc in range(DC):
                        nc.tensor.transpose(tp[:, dc, :],
                                            yn[:, dc * 128:(dc + 1) * 128], ident)
                    for dc in range(DC):
                        nc.vector.tensor_scalar(
                            out=yT_dst[:, dc, it * 128:(it + 1) * 128],
                            in0=tp[:, dc, :], scalar1=lng[gk][:, dc:dc + 1],
                            scalar2=lng[bk][:, dc:dc + 1], op0=OP.mult, op1=OP.add)

            # ---------- attention ----------
            def attention(prefix, pool_a, qT_own, kvT, kv_len, use_mask, cacc):
                nkt = kv_len // 128

                def emit_combine(oTt, wct, first):
                    # C^T partial for a finished head; PSUM from "s" tag so the
                    # next head's AV accumulators ("big") are never blocked.
                    for dc in range(DC):
                        pc = ps.tile([128, LQ], F32, name="ps_c", tag="s", bufs=3)
                        for ec in range(ECH):
                            nc.tensor.matmul(pc,
                                             lhsT=wct[:, ec, dc * 128:(dc + 1) * 128],
                                             rhs=oTt[:, ec, :],
                                             start=ec == 0, stop=ec == ECH - 1)
                        if first:
                            nc.vector.tensor_copy(out=cacc[:, dc, :], in_=pc)
                        else:
                            nc.vector.tensor_add(out=cacc[:, dc, :],
                                                 in0=cacc[:, dc, :], in1=pc)

                pend = None
                for h in range(H):
                    wq_t = pool_a.tile([128, DC, E], F32R, name="wq", tag="wq", bufs=2)
                    wk_t = pool_a.tile([128, DC, E], F32R, name="wk", tag="wk", bufs=1)
                    wv_t = pool_a.tile([128, DC, E], F32R, name="wv", tag="wv", bufs=1)
                    for dc in range(DC):
                        nc.sync.dma_start(out=wq_t[:, dc, :], in_=w_in[prefix + "_wqT"][h, dc])
                        nc.sync.dma_start(out=wk_t[:, dc, :], in_=w_in[prefix + "_wkT"][h, dc])
                        nc.sync.dma_start(out=wv_t[:, dc, :], in_=w_in[prefix + "_wvT"][h, dc])
                    wc_t = pool_a.tile([128, ECH, D], F32R, name="wc", tag="wc", bufs=2)
                    for ec in range(ECH):
                        nc.sync.dma_start(out=wc_t[:, ec, :],
                                          in_=w_in[prefix + "_wcT"][h * ECH + ec])
                    bv_t = pool_a.tile([128, E], F32R, name="bv", tag="bv", bufs=2)
                    nc.sync.dma_start(
                        out=bv_t,
                        in_=w_in[prefix + "_bv"][h:h + 1, :].to_broadcast((128, E)))

                    qT = pool_a.tile([128, ECH, LQ], F32R, name="qT", tag="qT", bufs=1)
                    kT = pool_a.tile([128, ECH, kv_len], F32R, name="kT", tag="kT", bufs=1)
                    for ec in range(ECH):
                        for lt in range(kv_len // 512):
                            acck = ps.tile([128, 512], F32, name="ps_k", tag="s", bufs=3)
                            for dc in range(DC):
                                nc.tensor.matmul(
                                    acck, lhsT=wk_t[:, dc, ec * 128:(ec + 1) * 128],
                                    rhs=kvT[:, dc, lt * 512:(lt + 1) * 512],
                                    start=dc == 0, stop=dc == DC - 1)
                            nc.scalar.activation(
                                out=kT[:, ec, lt * 512:(lt + 1) * 512], in_=acck,
                                func=AF.Identity,
                                bias=biask[prefix][:, h, ec:ec + 1], scale=1.0)
                    for ec in range(ECH):
                        accq = ps.tile([128, LQ], F32, name="ps_q", tag="s", bufs=3)
                        for dc in range(DC):
                            nc.tensor.matmul(accq,
                                             lhsT=wq_t[:, dc, ec * 128:(ec + 1) * 128],
                                             rhs=qT_own[:, dc, :],
                                             start=dc == 0, stop=dc == DC - 1)
                        nc.scalar.activation(
                            out=qT[:, ec, :], in_=accq, func=AF.Identity,
                            bias=biasq[prefix][:, h, ec:ec + 1], scale=1.0)

                    # streaming over key tiles: V jit, scores, exp, mask, sums, AV
                    sums = ps.tile([1, LQ], F32, name="ps_sum", tag="sum", bufs=1)
                    po = [ps.tile([128, LQ], F32, name=f"ps_o{ec}", tag="big", bufs=4)
                          for ec in range(ECH)]
                    for kt in range(nkt):
                        accv = ps.tile([128, E], F32, name="ps_v", tag="s", bufs=3)
                        for dc in range(DC):
                            nc.tensor.matmul(accv,
                                             lhsT=kvT[:, dc, kt * 128:(kt + 1) * 128],
                                             rhs=wv_t[:, dc, :],
                                             start=dc == 0, stop=dc == DC - 1)
                        vt = pool_a.tile([128, E], F32R, name="vt", tag="vt", bufs=2)
                        nc.vector.tensor_add(out=vt, in0=accv, in1=bv_t)

                        sc = ps.tile([128, LQ], F32, name="ps_s", tag="s", bufs=3)
                        for ec in range(ECH):
                            nc.tensor.matmul(sc,
                                             lhsT=kT[:, ec, kt * 128:(kt + 1) * 128],
                                             rhs=qT[:, ec, :],
                                             start=ec == 0, stop=ec == ECH - 1)
                        pt = pool_a.tile([128, LQ], F32R, name="pt", tag="pt", bufs=2)
                        nc.scalar.activation(out=pt, in_=sc, func=AF.Exp, scale=SCALE,
                                             bias=zero_t)
                        if use_mask:
                            nc.gpsimd.tensor_mul(out=pt, in0=pt, in1=mask_t[:, kt, :])
                        nc.tensor.matmul(sums, lhsT=ones_k, rhs=pt,
                                         start=kt == 0, stop=kt == nkt - 1)
                        for ec in range(ECH):
                            nc.tensor.matmul(po[ec],
                                             lhsT=vt[:, ec * 128:(ec + 1) * 128],
                                             rhs=pt, start=kt == 0, stop=kt == nkt - 1)
                        if debug and prefix == "sa" and h == 0:
                            dt1 = pool.tile([128, LQ], F32, name="dbg1", tag="dbg", bufs=2)
                            nc.vector.tensor_copy(out=dt1, in_=pt.bitcast(F32))
                            nc.sync.dma_start(out=dbg["dbg_pT"][:, kt, :], in_=dt1)

                    # previous head's combine fills the PE while this head's
                    # softmax-tail chain (DVE/ACT latency) resolves
                    if pend is not None:
                        emit_combine(*pend)

                    recip = pool_a.tile([1, LQ], F32R, name="recip", tag="recip", bufs=2)
                    nc.vector.reciprocal(out=recip, in_=sums)
                    rb_ps = ps.tile([128, LQ], F32, name="ps_rb", tag="s", bufs=3)
                    nc.tensor.matmul(rb_ps, lhsT=ones_r, rhs=recip, start=True, stop=True)
                    rb = pool_a.tile([128, LQ], F32, name="rb", tag="rb", bufs=2)
                    nc.vector.tensor_copy(out=rb, in_=rb_ps)

                    oT = pool_a.tile([128, ECH, LQ], F32R, name="oT", tag="oT", bufs=2)
                    for ec in range(ECH):
                        nc.vector.tensor_mul(out=oT[:, ec, :], in0=po[ec], in1=rb)
                    if debug and prefix == "sa" and h == 0:
                        for ec in range(ECH):
                            dt2 = pool.tile([128, LQ], F32, name="dbg2", tag="dbg", bufs=2)
                            nc.vector.tensor_copy(out=dt2, in_=oT[:, ec, :].bitcast(F32))
                            nc.sync.dma_start(out=dbg["dbg_oT"][:, ec, :], in_=dt2)
                    pend = (oT, wc_t, h == 0)
                emit_combine(*pend)

            # ---------- epilogue: xout(N) = base(N) + transpose(cacc + bc) ----------
            def epilogue(cacc, bcc_t, base_dram, base_sbuf, xout):
                for dc in range(DC):
                    nc.vector.tensor_scalar_add(out=cacc[:, dc, :], in0=cacc[:, dc, :],
                                                scalar1=bcc_t[:, dc:dc + 1])
                for qt in range(QT):
                    tp = ps.tile([128, DC, 128], F32, name="ps_ep", tag="s", bufs=3)
                    for dc in range(DC):
                        nc.tensor.transpose(tp[:, dc, :],
                                            cacc[:, dc, qt * 128:(qt + 1) * 128], ident)
                    tpf = tp.rearrange("p a b -> p (a b)")
                    if base_sbuf is None:
                        bt = pool.tile([128, D], F32, name="ep_b", tag="ln_x", bufs=2)
                        nc.sync.dma_start(out=bt, in_=base_dram[qt].bitcast(F32))
                        nc.vector.tensor_add(out=xout[:, qt, :], in0=tpf, in1=bt)
                    else:
                        nc.vector.tensor_add(out=xout[:, qt, :], in0=tpf,
                                             in1=base_sbuf[:, qt, :])

            # ================= pass =================
            ln_to_T(dec_full, LT, y1T, "ln1_gc", "ln1_bc")
            ln_to_T(dec_own, QT, y1To, "ln1_gc", "ln1_bc")
            if debug:
                for dc in range(DC):
                    for lt in range(2):
                        dt0 = pool.tile([128, 512], F32, name="dbg0", tag="dbg", bufs=2)
                        nc.vector.tensor_copy(
                            out=dt0, in_=y1T[:, dc, lt * 512:(lt + 1) * 512].bitcast(F32))
                        nc.sync.dma_start(
                            out=dbg["dbg_y1T"][:, dc, lt * 512:(lt + 1) * 512], in_=dt0)

            attn1 = tc.alloc_tile_pool(name="attn1", bufs=1)
            cacc1 = pool.tile([128, DC, LQ], F32, name="cacc1", tag="cacc")
            attention("sa", attn1, y1To, y1T, L, True, cacc1)
            epilogue(cacc1, biasc["sa"], dec_own, None, x1)
            attn1.release()
            sa_pool.release()
            ffn_w1a = tc.alloc_tile_pool(name="ffn_w1a", bufs=1)
            w1a_t = ffn_w1a.tile([128, DC, F // 2], F32R)
            for dc in range(DC):
                nc.sync.dma_start(out=w1a_t[:, dc, :], in_=ff_w1T[dc, :, :F // 2])
            if debug:
                for qt in range(QT):
                    nc.sync.dma_start(out=dbg["dbg_x1"][qt], in_=x1[:, qt, :])

            y2To = pool.tile([128, DC, LQ], F32R, name="y2To", tag="yTo")
            ln_to_T(None, QT, y2To, "ln2_gc", "ln2_bc", x_sbuf=x1)
            attn2 = tc.alloc_tile_pool(name="attn2", bufs=1)
            cacc2 = pool.tile([128, DC, LQ], F32, name="cacc2", tag="cacc")
            attention("ca", attn2, y2To, enc_t, L, False, cacc2)
            epilogue(cacc2, biasc["ca"], None, x1, x2)
            attn2.release()
            if debug:
                for qt in range(QT):
                    nc.sync.dma_start(out=dbg["dbg_x2"][qt], in_=x2[:, qt, :])

            y3To = pool.tile([128, DC, LQ], F32R, name="y3To", tag="yTo")
            ln_to_T(None, QT, y3To, "ln3_gc", "ln3_bc", x_sbuf=x2)

            # ---------- FFN ----------
            ffn = tc.alloc_tile_pool(name="ffn", bufs=1)
            w1b_t = ffn.tile([128, DC, F // 2], F32R)
            for dc in range(DC):
                nc.sync.dma_start(out=w1b_t[:, dc, :], in_=ff_w1T[dc, :, F // 2:])
            w2_t = ffn.tile([128, FC, D], F32R)
            for fc in range(FC):
                nc.sync.dma_start(out=w2_t[:, fc, :], in_=ff_w2T[fc])
            uT = ffn.tile([128, FC, LQ], F32R)
            for fc in range(FC):
                pu = ps.tile([128, LQ], F32, name="ps_u", tag="s", bufs=3)
                wsrc = w1a_t if fc < FC // 2 else w1b_t
                fcl = fc if fc < FC // 2 else fc - FC // 2
                for dc in range(DC):
                    nc.tensor.matmul(pu, lhsT=wsrc[:, dc, fcl * 128:(fcl + 1) * 128],
                                     rhs=y3To[:, dc, :], start=dc == 0, stop=dc == DC - 1)
                a = ffn.tile([128, LQ], F32, name="ff_a", tag="ff_a", bufs=2)
                nc.scalar.activation(out=a, in_=pu, func=AF.Identity,
                                     bias=b1t[:, fc:fc + 1], scale=1.0)
                mn = ffn.tile([128, LQ], F32, name="ff_m", tag="ff_m", bufs=2)
                nc.vector.tensor_scalar_min(out=mn, in0=a, scalar1=0.0)
                ex = ffn.tile([128, LQ], F32, name="ff_e", tag="ff_e", bufs=2)
                nc.scalar.activation(out=ex, in_=mn, func=AF.Exp, bias=zero_t)
                p1 = ffn.tile([128, LQ], F32, name="ff_p", tag="ff_p", bufs=2)
                nc.vector.tensor_scalar(out=p1, in0=a, scalar1=0.0, scalar2=-1.0,
                                        op0=OP.max, op1=OP.add)
                nc.vector.tensor_add(out=uT[:, fc, :], in0=ex, in1=p1)
            fT = ffn.tile([128, DC, LQ], F32)
            for dc in range(DC):
                pf = ps.tile([128, LQ], F32, name="ps_f", tag="big", bufs=4)
                for fc in range(FC):
                    nc.tensor.matmul(pf, lhsT=w2_t[:, fc, dc * 128:(dc + 1) * 128],
                                     rhs=uT[:, fc, :], start=fc == 0, stop=fc == FC - 1)
                nc.scalar.activation(out=fT[:, dc, :], in_=pf, func=AF.Identity,
                                     bias=b2t[:, dc:dc + 1], scale=1.0)
            for qt in range(QT):
                tp = ps.tile([128, DC, 128], F32, name="ps_ft", tag="s", bufs=3)
                for dc in range(DC):
                    nc.tensor.transpose(tp[:, dc, :],
                                        fT[:, dc, qt * 128:(qt + 1) * 128], ident)
                ot = ffn.tile([128, D], F32, name="out_t", tag="out_t", bufs=2)
                nc.vector.tensor_add(out=ot, in0=tp.rearrange("p a b -> p (a b)"),
                                     in1=x2[:, qt, :])
                nc.sync.dma_start(out=out[qt], in_=ot)
            ffn.release()
            ffn_w1a.release()
            pool.release()

        if n_iter > 0:
            with tc.For_i(0, n_iter, 1):
                body()
        else:
            body()

    nc.compile()
    return nc


# --------------------------------------------------------------------------
# host side
# --------------------------------------------------------------------------
def _prep_shared(params):
    P = {k: np.ascontiguousarray(np.asarray(v, dtype=np.float32))
         for k, v in params.items()}
    s = {}
    for p in ("sa", "ca"):
        s[p + "_wqT"] = np.ascontiguousarray(
            P[p + "_Wq"].transpose(0, 2, 1)).reshape(H, DC, 128, E)
        s[p + "_wkT"] = np.ascontiguousarray(
            P[p + "_Wk"].transpose(0, 2, 1)).reshape(H, DC, 128, E)
        s[p + "_wvT"] = np.ascontiguousarray(
            P[p + "_Wv"].transpose(0, 2, 1)).reshape(H, DC, 128, E)
        s[p + "_wcT"] = np.ascontiguousarray(P[p + "_Wc"].T).reshape(WCC, 128, D)
        s[p + "_bqc"] = np.ascontiguousarray(
            P[p + "_bq"].reshape(H, ECH, 128).transpose(2, 0, 1))
        s[p + "_bkc"] = np.ascontiguousarray(
            P[p + "_bk"].reshape(H, ECH, 128).transpose(2, 0, 1))
        s[p + "_bv"] = P[p + "_bv"]
        s[p + "_bcc"] = np.ascontiguousarray(P[p + "_bc"].reshape(DC, 128).T)
    s["ff_w1T"] = np.ascontiguousarray(P["ff_W1"].T).reshape(DC, 128, F)
    s["ff_w2T"] = np.ascontiguousarray(P["ff_W2"].T).reshape(FC, 128, D)
    s["ff_b1c"] = np.ascontiguousarray(P["ff_b1"].reshape(FC, 128).T)
    s["ff_b2c"] = np.ascontiguousarray(P["ff_b2"].reshape(DC, 128).T)
    for i in (1, 2, 3):
        s[f"ln{i}_gc"] = np.ascontiguousarray(P[f"ln{i}_g"].reshape(DC, 128).T)
        s[f"ln{i}_bc"] = np.ascontiguousarray(P[f"ln{i}_b"].reshape(DC, 128).T)
    return s


def _prep_percore(decoder_embeddings, encoder_embeddings):
    dec = np.asarray(decoder_embeddings, dtype=np.float32)
    enc = np.asarray(encoder_embeddings, dtype=np.float32)
    per = []
    k_global = (np.arange(128)[:, None] + 128 * np.arange(LT)[None, :])  # [128, LT]
    for c in range(NCORES):
        b, half = c // 2, c % 2
        q0 = half * LQ
        m = (k_global[:, :, None] <= (q0 + np.arange(LQ))[None, None, :])
        per.append({
            "dec_full": np.ascontiguousarray(dec[b].reshape(LT, 128, D)),
            "dec_own": np.ascontiguousarray(dec[b, q0:q0 + LQ].reshape(QT, 128, D)),
            "encT": np.ascontiguousarray(enc[b].T).reshape(DC, 128, L),
            "mask": np.ascontiguousarray(m.astype(np.float32)),
        })
    return per


def make_runner(nc):
    """Persistent jitted SPMD runner. Weights are replicated (no 8x concat)."""
    import jax
    from jax.sharding import Mesh, PartitionSpec
    from jax.experimental.shard_map import shard_map
    from concourse import mybir
    from concourse.bass2jax import (_bass_exec_p, install_neuronx_cc_hook,
                                    partition_id_tensor)
    install_neuronx_cc_hook()

    in_names, out_names, out_avals = [], [], []
    for alloc in nc.m.functions[0].allocations:
        if not isinstance(alloc, mybir.MemoryLocationSet):
            continue
        name = alloc.memorylocations[0].name
        if alloc.kind == "ExternalInput":
            in_names.append(name)
        elif alloc.kind == "ExternalOutput":
            out_names.append(name)
            out_avals.append(jax.core.ShapedArray(tuple(alloc.tensor_shape),
                                                  mybir.dt.np(alloc.dtype)))
    pname = nc.partition_id_tensor.name if nc.partition_id_tensor else None
    if pname:
        in_names.remove(pname)
    all_in = in_names + out_names + ([pname] if pname else [])

    def _body(*args):
        ops = list(args)
        if pname:
            ops.append(partition_id_tensor())
        return tuple(_bass_exec_p.bind(
            *ops, out_avals=tuple(out_avals), in_names=tuple(all_in),
            out_names=tuple(out_names), lowering_input_output_aliases=(),
            sim_require_finite=True, sim_require_nnan=True, nc=nc))

    mesh = Mesh(np.asarray(jax.devices()[:NCORES]), ("core",))
    in_specs = tuple(PartitionSpec("core") if n in PER_CORE_INPUTS else PartitionSpec()
                     for n in in_names) + (PartitionSpec("core"),) * len(out_names)
    out_specs = (PartitionSpec("core"),) * len(out_names)
    f = jax.jit(shard_map(_body, mesh=mesh, in_specs=in_specs, out_specs=out_specs,
                          check_rep=False), keep_unused=True)

    def run(shared, per_core, n_rep=1):
        import time
        args = []
        for n in in_names:
            if n in PER_CORE_INPUTS:
                args.append(np.concatenate([per_core[c][n] for c in range(NCORES)],
                                           axis=0))
            else:
                args.append(shared[n])
        for a in out_avals:
            args.append(np.zeros((NCORES * a.shape[0], *a.shape[1:]), a.dtype))
        args = [jax.device_put(a) for a in args]
        outs = f(*args)
        jax.block_until_ready(outs)
        walls = []
        for _ in range(max(0, n_rep - 1)):
            t0 = time.time()
            outs = f(*args)
            jax.block_until_ready(outs)
            walls.append(time.time() - t0)
        res = {nm: np.asarray(outs[i]).reshape(NCORES, *out_avals[i].shape)
               for i, nm in enumerate(out_names)}
        return res, walls

    return run


def _get(key, builder):
    if key not in _CACHE:
        _CACHE[key] = builder()
    return _CACHE[key]


def kernel(decoder_embeddings, encoder_embeddings, params):
    runner = _get("runner0", lambda: make_runner(_get("nc0", lambda: build_nc())))
    shared = _prep_shared(params)
    per_core = _prep_percore(decoder_embeddings, encoder_embeddings)
    res, _ = runner(shared, per_core)
    outs = res["out"]  # [8, QT, 128, D]
    full = np.empty((B, L, D), dtype=np.float32)
    for c in range(NCORES):
        b, half = c // 2, c % 2
        full[b, half * LQ:(half + 1) * LQ] = outs[c].reshape(LQ, D)
    return full
